# revision 36
# baseline (speedup 1.0000x reference)
"""FEDformer encoder layer on 8 TRN2 NeuronCores — batch-data-parallel Bass kernel.

Strategy (self-contained; shapes hardcoded):
  B=16,L=2048,D=512,H=8,E=64,M=64,DFF=2048; 8 cores x 2 batches each; no collectives.

  Math restructuring (validated against the jax reference):
   - rfft+mode-gather == x @ Fcat where Fcat[l, 0:64]=cos(2*pi*k_j*l/L),
     Fcat[l, 64:128]=-sin(...), k_j = mode_index.
   - Wq/Wo commute with the DFT -> applied in mode space. k/v projections are
     dead code in the reference.
   - irfft of a spectrum with only the selected modes == P @ C2S2.
   - The Fourier branch contributes ~1e-5 absolute to an O(1) output, so the
     whole branch runs in fp8/bf16 (WPK pre-scaled by 2^17 on host; 2^-17
     folded into the iDFT matrices).
   - series-decomp: K=2 softmax == sigmoid of weight/bias deltas; moving
     averages via fp32 cumsum over a replicate-padded tile + shifted
     subtracts (pads baked into the padded layout; no edge fixups).
   - decomp1 split trick: u = (x+bo) + y with y = pcat @ C2S2 linear, so
     S13(u) = S13(x+bo) + pcat @ C13 (C13 = window-summed C2S2, host-made).
     The x-side scans/diffs run at kernel start, hiding the WPK DMA.
   - FFN in fp8e4 DoubleRow (weights x16 host-side; 1/16 folded into the
     gelu input scale and the final residual add).
   - bo folded into the host-prepared x (XTB = (x+bo)^T).

  Layout: device works feature-major ([D, Lpad]) in bf16; token-major fp8
  copy (XBF) only for the DFT.
"""

import numpy as np

B, L, D, H, M, DFF = 16, 2048, 512, 8, 64, 2048
E = D // H
NC_ = 8
BLOC = B // NC_          # batches per core
MEXT = 2 * M             # re|im rows
NDC = D // 128           # 4 feature tiles
NFF = DFF // 128         # 16 dff tiles
NLC = L // 128           # 16 token chunks of 128
NTC = L // 512           # 4 token chunks of 512
PADL = 13                # left replicate pad (cumsum needs one extra)
PADR = 12
LP = 2080                # PADL + L + PADR + 7 spare zeros
D0 = PADL                # data column offset in padded tiles
WPKSH = float(2 ** 17)   # fp8 scale for Fourier weights
FFNS = 16.0              # fp8 scale for FFN weights

_prog_cache = {}
_fixn = [0]


def _fix_sync_waits(nc, max_waits=1, max_updates=4):
    """Split >max sem-waits/updates per instruction onto adjacent nops.

    The AWS neuronx-cc walrus rejects instructions carrying too many sync
    commands ("Too many sync wait commands"); Tile's tail drain aggregates one
    wait per outstanding semaphore. Engine-order execution makes the split
    semantically identical.
    """
    import concourse.mybir as mybir

    for f in nc.m.functions:
        for bb in f.blocks:
            insts = bb.instructions
            i = 0
            while i < len(insts):
                ins = insts[i]
                si = ins.sync_info
                if si is not None and si.on_wait and len(si.on_wait) > max_waits:
                    waits = list(si.on_wait)
                    si.on_wait = waits[-max_waits:]
                    rest = waits[:-max_waits]
                    chunks = [rest[j:j + max_waits]
                              for j in range(0, len(rest), max_waits)]
                    for c in reversed(chunks):
                        _fixn[0] += 1
                        nop = mybir.InstNoOp(name=f"I-fixw-{_fixn[0]}", ins=[], outs=[])
                        nop.engine = ins.engine
                        nop.sync_info = mybir.SyncInfo(on_wait=c, on_update=[])
                        insts.insert(i, nop)
                        i += 1
                if si is not None and si.on_update and len(si.on_update) > max_updates:
                    ups = list(si.on_update)
                    si.on_update = ups[:max_updates]
                    rest = ups[max_updates:]
                    chunks = [rest[j:j + max_updates]
                              for j in range(0, len(rest), max_updates)]
                    for c in chunks:
                        _fixn[0] += 1
                        nop = mybir.InstNoOp(name=f"I-fixu-{_fixn[0]}", ins=[], outs=[])
                        nop.engine = ins.engine
                        nop.sync_info = mybir.SyncInfo(on_wait=[], on_update=c)
                        insts.insert(i + 1, nop)
                        i += 1
                i += 1


def _build_program(need_bq, j0, fix=True):
    import concourse.bass as bass
    import concourse.mybir as mybir
    from concourse.tile import TileContext

    F32 = mybir.dt.float32
    BF16 = mybir.dt.bfloat16
    FP8 = mybir.dt.float8e4
    AF = mybir.ActivationFunctionType
    OP = mybir.AluOpType
    DR = mybir.MatmulPerfMode.DoubleRow

    nc = bass.Bass()

    # ---- DRAM I/O ----
    XTB = nc.dram_tensor("XTB", [BLOC, D, LP], BF16, kind="ExternalInput")
    XBF = nc.dram_tensor("XBF", [BLOC, 128, NLC * D], FP8, kind="ExternalInput")
    FCT = nc.dram_tensor("FCT", [128, NLC * 128], FP8, kind="ExternalInput")
    C2S2 = nc.dram_tensor("C2S2", [128, L], BF16, kind="ExternalInput")
    C13 = nc.dram_tensor("C13", [128, L], BF16, kind="ExternalInput")
    C25 = nc.dram_tensor("C25", [128, L], BF16, kind="ExternalInput")
    WQT = nc.dram_tensor("WQT", [D, D], BF16, kind="ExternalInput")
    WOT = nc.dram_tensor("WOT", [D, D], BF16, kind="ExternalInput")
    WPK = nc.dram_tensor("WPK", [H, 128, M * 128], FP8, kind="ExternalInput")
    W1T = nc.dram_tensor("W1T", [128, NDC, DFF], FP8, kind="ExternalInput")
    W2T = nc.dram_tensor("W2T", [128, NFF, D], FP8, kind="ExternalInput")
    EYE = nc.dram_tensor("EYE", [128, 128], BF16, kind="ExternalInput")
    BQ4 = nc.dram_tensor("BQ4", [128, NDC], F32, kind="ExternalInput")
    DECS = nc.dram_tensor("DECS", [128, 4], F32, kind="ExternalInput")
    F16 = mybir.dt.float16
    OUT_T = nc.dram_tensor("OUT_T", [BLOC, D, L], F16, kind="ExternalOutput")

    with TileContext(nc) as tc:
        # ---------- persistent pools (LIFO: wpkp/fr close after fourier,
        # ffnw after the FFN, the rest at the end) ----------
        cst_cm = tc.tile_pool(name="cst", bufs=1)
        cst = cst_cm.__enter__()
        main_cm = tc.tile_pool(name="main", bufs=1)
        mainp = main_cm.__enter__()
        ear_cm = tc.tile_pool(name="ear", bufs=1)
        ear = ear_cm.__enter__()
        ffnw_cm = tc.tile_pool(name="ffnw", bufs=1)
        ffnw = ffnw_cm.__enter__()
        fr_cm = tc.tile_pool(name="fr", bufs=1)
        fr = fr_cm.__enter__()

        # DFT inputs first (DFT is the head of the dependency chain), then the
        # first WPK chunks (mode-mix stream), then x, then later-used consts.
        fct = cst.tile([128, NLC * 128], FP8, name="fct")
        nc.sync.dma_start(out=fct[:], in_=FCT[:])
        xbfs = [cst.tile([128, NLC * D], FP8, name=f"xbf{b}", tag=f"xbf{b}")
                for b in range(BLOC)]
        for b in range(BLOC):
            nc.sync.dma_start(out=xbfs[b][:], in_=XBF[b])
        wqt = [cst.tile([128, D], BF16, name=f"wqt{i}") for i in range(NDC)]
        for i in range(NDC):
            nc.sync.dma_start(out=wqt[i][:], in_=WQT[i * 128:(i + 1) * 128, :])

        wpk_cm = tc.tile_pool(name="wpkp", bufs=4)
        wpkp = wpk_cm.__enter__()
        wpk_pre = []
        for q in range(4):  # first head's chunks, prefetched from t=0
            t_ = wpkp.tile([128, 16 * 128], FP8, name=f"wpk0_{q}", tag="wpk")
            nc.sync.dma_start(out=t_[:], in_=WPK[0][:, q * 2048:(q + 1) * 2048])
            wpk_pre.append(t_)

        # main activation tiles: (x+bo) -> u -> r1 -> v, in place, bf16 padded
        mt = [[mainp.tile([128, LP], BF16, name=f"m_{b}_{dc}")
               for dc in range(NDC)] for b in range(BLOC)]
        for b in range(BLOC):
            for dc in range(NDC):
                nc.sync.dma_start(out=mt[b][dc][:],
                                  in_=XTB[b, dc * 128:(dc + 1) * 128, :])

        c2s2 = cst.tile([128, L], BF16, name="c2s2")
        c13 = cst.tile([128, L], BF16, name="c13")
        c25 = cst.tile([128, L], BF16, name="c25")
        nc.sync.dma_start(out=c2s2[:], in_=C2S2[:])
        nc.sync.dma_start(out=c13[:], in_=C13[:])
        nc.sync.dma_start(out=c25[:], in_=C25[:])
        wot = [cst.tile([128, D], BF16, name=f"wot{i}") for i in range(NDC)]
        for i in range(NDC):
            nc.sync.dma_start(out=wot[i][:], in_=WOT[i * 128:(i + 1) * 128, :])
        eye = cst.tile([128, 128], BF16, name="eye")
        nc.sync.dma_start(out=eye[:], in_=EYE[:])
        decs = cst.tile([128, 4], F32, name="decs")
        nc.sync.dma_start(out=decs[:], in_=DECS[:])
        ones13 = cst.tile([128, PADL], BF16, name="ones13")
        nc.vector.memset(ones13[:], 1.0)
        bq4 = None
        if need_bq:
            bq4 = cst.tile([128, NDC], F32, name="bq4")
            nc.sync.dma_start(out=bq4[:], in_=BQ4[:])

        # early pool tiles: rotating cumsums + per-tile windowed sums
        NCS = 2
        def cs_tile(i):
            return ear.tile([128, LP], F32, name="cs", tag=f"cs{i % NCS}")
        def stage_tile(i):
            return ear.tile([128, L], F16, name="stg", tag=f"stg{i % 2}")
        def scr_tile(i):
            return ear.tile([128, L], F32, name="scr", tag="scr0")
        m13x = [[ear.tile([128, L], BF16, name=f"m13x{b}{dc}", tag=f"m13x{b}{dc}")
                 for dc in range(NDC)] for b in range(BLOC)]
        m25x = [[ear.tile([128, L], BF16, name=f"m25x{b}{dc}", tag=f"m25x{b}{dc}")
                 for dc in range(NDC)] for b in range(BLOC)]

        # ---------- early: scans + windowed diffs of (x+bo), fills the ----
        # ---------- window where the PE waits on the WPK weight stream ----
        csi = 0
        for b in range(BLOC):
            for dc in range(NDC):
                cs = cs_tile(csi)
                scr = scr_tile(csi)
                csi += 1
                u = mt[b][dc]
                nc.vector.tensor_tensor_scan(cs[:], u[:], u[:], 0.0,
                                             OP.add, OP.bypass)
                # S13(t) = cs[t+19] - cs[t+6]; S25(t) = cs[t+25] - cs[t]
                nc.vector.tensor_tensor(m13x[b][dc][:], cs[:, 19:2067],
                                        cs[:, 6:2054], OP.subtract)
                nc.scalar.mul(m13x[b][dc][:], m13x[b][dc][:], 1.0 / 13.0)
                nc.gpsimd.tensor_tensor(scr[:], cs[:, 25:2073],
                                        cs[:, 0:2048], OP.subtract)
                nc.scalar.mul(m25x[b][dc][:], scr[:], 1.0 / 25.0)

        # ---------- Fourier branch (fp8/bf16) ----------
        with tc.tile_pool(name="frp", bufs=2, space="PSUM") as frp:
            qt = [[None] * NDC for _ in range(BLOC)]
            for b in range(BLOC):
                xbf = xbfs[b]
                # DFT: xselT[d, m-ext] = sum_l x[l, d] * Fcat[l, m-ext]
                xselT = fr.tile([128, NDC * 128], BF16, name=f"xselT{b}",
                                tag=f"xselT{b}")
                for dc in range(NDC):
                    ps = frp.tile([128, 128], F32, name="psA", tag="psA")
                    for lc in range(NLC):
                        nc.tensor.matmul(
                            ps[:],
                            xbf[:, lc * D + dc * 128: lc * D + (dc + 1) * 128],
                            fct[:, lc * 128:(lc + 1) * 128],
                            start=(lc == 0), stop=(lc == NLC - 1))
                    nc.scalar.copy(xselT[:, dc * 128:(dc + 1) * 128], ps[:])
                # q-projection in mode space (fp8 out for the mode mix)
                for do in range(NDC):
                    qt[b][do] = fr.tile([128, 128], FP8, name=f"qt{b}_{do}",
                                        tag=f"qt{b}_{do}")
                    ps = frp.tile([128, 128], F32, name="psQ", tag="psA")
                    for dc in range(NDC):
                        nc.tensor.matmul(
                            ps[:], wqt[dc][:, do * 128:(do + 1) * 128],
                            xselT[:, dc * 128:(dc + 1) * 128],
                            start=(dc == 0), stop=(dc == NDC - 1))
                    if need_bq:
                        nc.vector.tensor_tensor(
                            ps[:, j0:j0 + 1], ps[:, j0:j0 + 1],
                            bq4[:, do:do + 1], OP.add)
                    nc.scalar.copy(qt[b][do][:], ps[:])

            # mode mix: RH_h rows 0:64 = Qre, 64:128 = Qim; col = 2m + b
            rh = [fr.tile([128, 128], FP8, name=f"rh{h}", tag=f"rh{h}")
                  for h in range(H)]
            for h in range(H):
                src_do, r0 = h // 2, (h % 2) * 64
                for b in range(BLOC):
                    rhv = rh[h].rearrange("p (m t) -> p m t", t=2)
                    nc.scalar.copy(rhv[0:64, :, b], qt[b][src_do][r0:r0 + 64, 0:64])
                    nc.scalar.copy(rhv[64:128, :, b], qt[b][src_do][r0:r0 + 64, 64:128])
            otre = [[fr.tile([128, M], BF16, name=f"otre{b}_{dc}", tag=f"otre{b}{dc}")
                     for dc in range(NDC)] for b in range(BLOC)]
            otim = [[fr.tile([128, M], BF16, name=f"otim{b}_{dc}", tag=f"otim{b}{dc}")
                     for dc in range(NDC)] for b in range(BLOC)]
            for h in range(H):
                psm = frp.tile([128, 128], F32, name="psM", tag="psM")
                for q in range(4):
                    if h == 0:
                        wpk_q = wpk_pre[q]
                    else:
                        wpk_q = wpkp.tile([128, 16 * 128], FP8,
                                          name=f"wpk{h}_{q}", tag="wpk")
                        nc.sync.dma_start(out=wpk_q[:],
                                          in_=WPK[h][:, q * 2048:(q + 1) * 2048])
                    for mq in range(16):
                        m = q * 16 + mq
                        nc.tensor.matmul(
                            psm[:, 2 * m:2 * m + 2],
                            wpk_q[:, mq * 128:(mq + 1) * 128],
                            rh[h][:, 2 * m:2 * m + 2],
                            start=True, stop=True)
                psv = psm.rearrange("p (m t) -> p m t", t=2)
                dc, r0 = h // 2, (h % 2) * 64
                for b in range(BLOC):
                    nc.scalar.copy(otre[b][dc][r0:r0 + 64, :], psv[0:64, :, b])
                    nc.scalar.copy(otim[b][dc][r0:r0 + 64, :], psv[64:128, :, b])

            # Wo projection in mode space, then transpose into pcat_b
            pcat = [fr.tile([128, D], BF16, name=f"pcat{b}", tag=f"pcat{b}")
                    for b in range(BLOC)]
            for b in range(BLOC):
                for ro, ot in ((0, otre[b]), (64, otim[b])):
                    for do in range(NDC):
                        ps = frp.tile([128, M], F32, name="psP", tag="psP")
                        for dc in range(NDC):
                            nc.tensor.matmul(
                                ps[:], wot[dc][:, do * 128:(do + 1) * 128],
                                ot[dc][:], start=(dc == 0), stop=(dc == NDC - 1))
                        pp = fr.tile([128, M], BF16, name=f"pp{ro}_{do}", tag="pp")
                        nc.scalar.copy(pp[:], ps[:])
                        pst = frp.tile([M, 128], BF16, name="psT", tag="psT")
                        nc.tensor.transpose(pst[:], pp[:], eye[:])
                        nc.scalar.copy(pcat[b][ro:ro + 64, do * 128:(do + 1) * 128],
                                       pst[:])

        # FFN weights arrive while decomp1 runs
        w1t = ffnw.tile([128, NDC, DFF], FP8, name="w1t")
        nc.sync.dma_start(out=w1t[:], in_=W1T[:])
        w2t = ffnw.tile([128, NFF, D], FP8, name="w2t")
        nc.sync.dma_start(out=w2t[:], in_=W2T[:])

        # ---------- iDFT + decomp1 late combine ----------
        # per (b,dc,t4): psy=(y + x) ; ps13=(Y13'+m13x) ; ps25=(Y25'+m25x)
        # u = copy(psy) ; g=sig(u) ; h=1-g ; r = u - ps13*g - ps25*h
        dl_cm = tc.tile_pool(name="dl", bufs=2)
        dl = dl_cm.__enter__()
        psy_cm = tc.tile_pool(name="psy", bufs=2, space="PSUM")
        psyp = psy_cm.__enter__()
        for b in range(BLOC):
            for dc in range(NDC):
                dcb = slice(dc * 128, (dc + 1) * 128)
                for t4 in range(NTC):
                    ts_ = slice(t4 * 512, (t4 + 1) * 512)
                    mts = mt[b][dc][:, D0 + t4 * 512: D0 + (t4 + 1) * 512]
                    psy = psyp.tile([128, 512], F32, name="psy", tag="psy")
                    nc.tensor.matmul(psy[:], pcat[b][:, dcb], c2s2[:, ts_],
                                     start=True, stop=False)
                    nc.tensor.matmul(psy[:], eye[:], mts,
                                     start=False, stop=True)
                    ps13 = psyp.tile([128, 512], F32, name="ps13", tag="ps13")
                    nc.tensor.matmul(ps13[:], pcat[b][:, dcb], c13[:, ts_],
                                     start=True, stop=False)
                    nc.tensor.matmul(ps13[:], eye[:], m13x[b][dc][:, ts_],
                                     start=False, stop=True)
                    ps25 = psyp.tile([128, 512], F32, name="ps25", tag="ps25")
                    nc.tensor.matmul(ps25[:], pcat[b][:, dcb], c25[:, ts_],
                                     start=True, stop=False)
                    nc.tensor.matmul(ps25[:], eye[:], m25x[b][dc][:, ts_],
                                     start=False, stop=True)
                    # element combine: r = u - ma25 - g*(ma13 - ma25)
                    gt = dl.tile([128, 512], BF16, name="gt", tag="gt")
                    m2 = dl.tile([128, 512], BF16, name="m2", tag="m2")
                    dx = dl.tile([128, 512], BF16, name="dx", tag="dx")
                    ft = dl.tile([128, 512], BF16, name="ft", tag="ft")
                    nc.scalar.copy(mts, psy[:])                      # u (bf16)
                    nc.scalar.activation(gt[:], mts, AF.Sigmoid,
                                         scale=decs[:, 0:1], bias=decs[:, 1:2])
                    nc.scalar.copy(m2[:], ps25[:])                   # ma25 (bf16)
                    nc.vector.tensor_tensor(dx[:], ps13[:], m2[:], OP.subtract)
                    nc.vector.tensor_tensor(dx[:], dx[:], gt[:], OP.mult)
                    nc.gpsimd.tensor_tensor(ft[:], mts, m2[:], OP.subtract)
                    nc.vector.tensor_tensor(mts, ft[:], dx[:], OP.subtract)
        psy_cm.__exit__(None, None, None)
        dl_cm.__exit__(None, None, None)
        wpk_cm.__exit__(None, None, None)
        fr_cm.__exit__(None, None, None)

        # ---------- FFN (fp8 DoubleRow) + decomp2 ----------
        # Engine plan: FFN(b0) element ops on DVE; decomp2(b0) on DVE+ACT
        # (issued between the two FFN batches, overlapping FFN(b1) on PE);
        # FFN(b1) element ops on ACT+Pool; decomp2(b1) split DVE+Pool.
        def pass2(b, dc, sidx):
            """v (mt, padded bf16) -> series-decomp residual -> f16 stage -> DMA."""
            u = mt[b][dc]
            # refresh replicate pads from v (fp32 edge columns for the scalar op)
            ec = ear.tile([128, 2], F32, name="ec", tag=f"ec{sidx % 2}")
            nc.vector.tensor_copy(ec[:, 0:1], u[:, D0:D0 + 1])
            nc.vector.tensor_copy(ec[:, 1:2], u[:, D0 + L - 1:D0 + L])
            nc.vector.tensor_scalar_mul(u[:, 0:D0], ones13[:], ec[:, 0:1])
            nc.vector.tensor_scalar_mul(u[:, D0 + L:D0 + L + PADR],
                                        ones13[:, 0:PADR], ec[:, 1:2])
            cs = cs_tile(sidx)
            nc.vector.tensor_tensor_scan(cs[:], u[:], u[:], 0.0, OP.add, OP.bypass)
            # reuse m-tile storage of this (b,dc) + the sibling batch's tiles
            d13 = ear.tile([128, L], BF16, name="d13", tag=f"m13x{b}{dc}")
            m25 = ear.tile([128, L], BF16, name="m25", tag=f"m25x{b}{dc}")
            ob = 1 - b
            gt = ear.tile([128, L], BF16, name="gt2", tag=f"m13x{ob}{dc}")
            ft = ear.tile([128, L], BF16, name="ft2", tag=f"m25x{ob}{dc}")
            scr = scr_tile(sidx)
            nc.vector.tensor_tensor(d13[:], cs[:, 19:2067], cs[:, 6:2054],
                                    OP.subtract)
            nc.gpsimd.tensor_tensor(scr[:], cs[:, 25:2073], cs[:, 0:2048],
                                    OP.subtract)
            nc.scalar.mul(m25[:], scr[:], 1.0 / 25.0)
            ud = u[:, D0:D0 + L]
            nc.scalar.activation(gt[:], ud, AF.Sigmoid,
                                 scale=decs[:, 2:3], bias=decs[:, 3:4])
            # r = v - m25 - g*(d13/13 - m25)
            nc.vector.scalar_tensor_tensor(d13[:], d13[:], 1.0 / 13.0, m25[:],
                                           OP.mult, OP.subtract)
            nc.vector.tensor_tensor(d13[:], d13[:], gt[:], OP.mult)
            nc.gpsimd.tensor_tensor(ft[:], ud, m25[:], OP.subtract)
            stg = stage_tile(sidx)
            nc.vector.tensor_tensor(stg[:], ft[:], d13[:], OP.subtract)
            nc.sync.dma_start(out=OUT_T[b, dc * 128:(dc + 1) * 128, :],
                              in_=stg[:])

        with tc.tile_pool(name="ffa", bufs=2) as ffa, \
             tc.tile_pool(name="ffb", bufs=1) as ffb, \
             tc.tile_pool(name="gqp", bufs=1) as gqp, \
             tc.tile_pool(name="tmq", bufs=2) as tmq, \
             tc.tile_pool(name="pshp", bufs=3, space="PSUM") as pshp, \
             tc.tile_pool(name="psfp", bufs=1, space="PSUM") as psfp:
            r1b1 = [ffb.tile([128, NDC, 512], FP8, name=f"r1b1_{t4}",
                             tag=f"r1b1_{t4}") for t4 in range(NTC)]

            def ffn_t4(b, t4):
                if b == 0:
                    r1c = ffa.tile([128, NDC, 512], FP8, name="r1c", tag="r1c")
                    for dc in range(NDC):
                        src = mt[b][dc][:, D0 + t4 * 512: D0 + (t4 + 1) * 512]
                        nc.vector.tensor_copy(r1c[:, dc, :], src)
                else:
                    r1c = r1b1[t4]
                gq = gqp.tile([128, NFF, 512], FP8, name="gq", tag="gq")
                for ff in range(NFF):
                    psh = pshp.tile([128, 512], F32, name="psH", tag="psH")
                    nc.tensor.matmul(psh[:], w1t[:, 0:2, ff * 128:(ff + 1) * 128],
                                     r1c[:, 0:2, :], start=True, stop=False,
                                     perf_mode=DR)
                    nc.tensor.matmul(psh[:], w1t[:, 2:4, ff * 128:(ff + 1) * 128],
                                     r1c[:, 2:4, :], start=False, stop=True,
                                     perf_mode=DR)
                    nc.scalar.activation(gq[:, ff, :], psh[:], AF.Gelu,
                                         scale=1.0 / FFNS)
                psf = [psfp.tile([128, 512], F32, name=f"psF{do}", tag=f"psF{do}")
                       for do in range(NDC)]
                for do in range(NDC):
                    for sp in range(0, NFF, 2):
                        nc.tensor.matmul(
                            psf[do][:], w2t[:, sp:sp + 2, do * 128:(do + 1) * 128],
                            gq[:, sp:sp + 2, :], start=(sp == 0),
                            stop=(sp == NFF - 2), perf_mode=DR)
                for do in range(NDC):
                    sl = mt[b][do][:, D0 + t4 * 512: D0 + (t4 + 1) * 512]
                    if b == 0:
                        nc.vector.scalar_tensor_tensor(
                            sl, psf[do][:], 1.0 / FFNS, sl, OP.mult, OP.add)
                    else:
                        # ACT drains psf, Pool adds (keeps DVE on decomp2(b0))
                        tm = tmq.tile([128, 512], BF16, name="tm", tag="tm")
                        nc.scalar.mul(tm[:], psf[do][:], 1.0 / FFNS)
                        nc.gpsimd.tensor_tensor(sl, sl, tm[:], OP.add)

            for t4 in range(NTC):
                ffn_t4(0, t4)
            # batch-1 FFN inputs cast on ACT (keeps DVE free for decomp2(b0))
            for t4 in range(NTC):
                for dc in range(NDC):
                    src = mt[1][dc][:, D0 + t4 * 512: D0 + (t4 + 1) * 512]
                    nc.scalar.copy(r1b1[t4][:, dc, :], src)
            # interleave: FFN(b1) on PE/ACT/Pool while decomp2(b0) runs on DVE
            for t4 in range(NTC):
                ffn_t4(1, t4)
                pass2(0, t4, t4)
            for dc in range(NDC):
                pass2(1, dc, NDC + dc)

        ffnw_cm.__exit__(None, None, None)
        ear_cm.__exit__(None, None, None)
        main_cm.__exit__(None, None, None)
        cst_cm.__exit__(None, None, None)

    if fix:
        _fix_sync_waits(nc)
    return nc


def _host_prep(inputs):
    import ml_dtypes
    bf16 = ml_dtypes.bfloat16
    fp8 = ml_dtypes.float8_e4m3
    x = np.asarray(inputs["x"], np.float32)
    bo = np.asarray(inputs["bo"], np.float32)
    modes = np.asarray(inputs["mode_index"]).astype(np.int64)
    l = np.arange(L, dtype=np.float64)
    ang = 2.0 * np.pi * np.outer(l, modes.astype(np.float64)) / L
    FC = np.concatenate([np.cos(ang), -np.sin(ang)], axis=1)          # [L, 128]
    m_out = np.arange(M, dtype=np.float64)
    w = np.where(m_out == 0, 1.0, 2.0) / L
    ang2 = 2.0 * np.pi * np.outer(m_out, l) / L
    C2 = np.concatenate([w[:, None] * np.cos(ang2),
                         w[:, None] * -np.sin(ang2)], axis=0)         # [128, L]
    C2 = C2 / WPKSH                                      # fp8 WPK compensation
    # replicate-clamped window sums of C2 (the y-side of decomp1 split)
    idx = np.arange(L)
    C13w = np.zeros_like(C2)
    for j in range(-6, 7):
        C13w += C2[:, np.clip(idx + j, 0, L - 1)]
    C13w /= 13.0
    C25w = np.zeros_like(C2)
    for j in range(-12, 13):
        C25w += C2[:, np.clip(idx + j, 0, L - 1)]
    C25w /= 25.0

    FCT = FC.reshape(NLC, 128, 128).transpose(1, 0, 2).reshape(128, NLC * 128)

    wr = np.asarray(inputs["four_wr"], np.float64)   # [H, E, O, M]
    wi = np.asarray(inputs["four_wi"], np.float64)
    wpk = np.zeros((H, M, 128, 128), np.float64)
    wpk[:, :, 0:64, 0:64] = wr.transpose(0, 3, 1, 2)
    wpk[:, :, 0:64, 64:128] = wi.transpose(0, 3, 1, 2)
    wpk[:, :, 64:128, 0:64] = -wi.transpose(0, 3, 1, 2)
    wpk[:, :, 64:128, 64:128] = wr.transpose(0, 3, 1, 2)
    WPKh = (wpk.transpose(0, 2, 1, 3).reshape(H, 128, M * 128)) * WPKSH

    dec1_w = np.asarray(inputs["dec1_w"], np.float64)
    dec1_b = np.asarray(inputs["dec1_b"], np.float64)
    dec2_w = np.asarray(inputs["dec2_w"], np.float64)
    dec2_b = np.asarray(inputs["dec2_b"], np.float64)
    decs = np.zeros((128, 4), np.float32)
    decs[:, 0] = dec1_w[0] - dec1_w[1]
    decs[:, 1] = dec1_b[0] - dec1_b[1]
    decs[:, 2] = dec2_w[0] - dec2_w[1]
    decs[:, 3] = dec2_b[0] - dec2_b[1]

    bq = np.asarray(inputs["bq"], np.float32)
    zero_pos = np.nonzero(modes == 0)[0]
    need_bq = bool(len(zero_pos)) and bool(np.any(bq != 0))
    j0 = int(zero_pos[0]) if need_bq else 0
    BQ4 = np.ascontiguousarray((L * bq).reshape(NDC, 128).T).astype(np.float32)

    # FFN weights: [128, S, F] fp8 with k-subtile interleave, x16
    w1 = np.asarray(inputs["conv1_w"], np.float32)   # [DFF, D]
    w2 = np.asarray(inputs["conv2_w"], np.float32)   # [D, DFF]
    W1T = (w1.T.reshape(NDC, 128, DFF) * FFNS).astype(fp8)          # [s,p,f]
    W1T = np.ascontiguousarray(W1T.transpose(1, 0, 2))              # [128,s,f]
    W2T = (w2.T.reshape(NFF, 128, D) * FFNS).astype(fp8)
    W2T = np.ascontiguousarray(W2T.transpose(1, 0, 2))

    shared = {
        "FCT": FCT.astype(fp8),
        "C2S2": C2.astype(bf16),
        "C13": C13w.astype(bf16),
        "C25": C25w.astype(bf16),
        "WQT": np.ascontiguousarray(np.asarray(inputs["Wq"], np.float32).T).astype(bf16),
        "WOT": np.ascontiguousarray(np.asarray(inputs["Wo"], np.float32).T).astype(bf16),
        "WPK": WPKh.astype(fp8),
        "W1T": W1T, "W2T": W2T,
        "EYE": np.eye(128, dtype=np.float32).astype(bf16),
        "BQ4": BQ4, "DECS": decs,
    }
    in_maps = []
    for c in range(NC_):
        xl = x[c * BLOC:(c + 1) * BLOC]                       # [2, L, D]
        xt = (xl + bo[None, None, :]).transpose(0, 2, 1)      # [2, D, L]
        xtp = np.zeros((BLOC, D, LP), np.float32)
        xtp[:, :, D0:D0 + L] = xt
        xtp[:, :, 0:D0] = xt[:, :, 0:1]
        xtp[:, :, D0 + L:D0 + L + PADR] = xt[:, :, L - 1:L]
        xbf = xl.astype(fp8)                                  # [2, L, D]
        XBFc = np.ascontiguousarray(
            xbf.reshape(BLOC, NLC, 128, D).transpose(0, 2, 1, 3)
        ).reshape(BLOC, 128, NLC * D)
        im = dict(shared)
        im["XTB"] = xtp.astype(bf16)
        im["XBF"] = XBFc
        in_maps.append(im)
    return in_maps, need_bq, j0


def kernel(**inputs):
    from concourse.bass_utils import run_bass_kernel_spmd

    in_maps, need_bq, j0 = _host_prep(inputs)
    key = (need_bq, j0)
    if key not in _prog_cache:
        _prog_cache[key] = _build_program(need_bq, j0)
    nc = _prog_cache[key]
    res = run_bass_kernel_spmd(nc, in_maps, core_ids=list(range(NC_)))
    outs = []
    for c in range(NC_):
        ot = np.asarray(res.results[c]["OUT_T"])              # [2, D, L]
        outs.append(np.ascontiguousarray(ot.transpose(0, 2, 1)))
    return np.concatenate(outs, axis=0).astype(np.float32)


# revision 42
# speedup vs baseline: 1.1635x; 1.1635x over previous
"""FEDformer encoder layer on 8 TRN2 NeuronCores — batch-data-parallel Bass kernel.

Strategy (self-contained; shapes hardcoded):
  B=16,L=2048,D=512,H=8,E=64,M=64,DFF=2048; 8 cores x 2 batches each; no collectives.

  Math restructuring (validated against the jax reference):
   - rfft+mode-gather == x @ Fcat where Fcat[l, 0:64]=cos(2*pi*k_j*l/L),
     Fcat[l, 64:128]=-sin(...), k_j = mode_index.
   - Wq/Wo commute with the DFT -> applied in mode space. k/v projections are
     dead code in the reference.
   - irfft of a spectrum with only the selected modes == P @ C2S2.
   - The Fourier branch contributes ~1e-5 absolute to an O(1) output, so the
     whole branch runs in fp8/bf16 (WPK pre-scaled by 2^17 on host; 2^-17
     folded into the iDFT matrices).
   - series-decomp: K=2 softmax == sigmoid of weight/bias deltas; moving
     averages via fp32 cumsum over a replicate-padded tile + shifted
     subtracts (pads baked into the padded layout; no edge fixups).
   - decomp1 split trick: u = (x+bo) + y with y = pcat @ C2S2 linear, so
     S13(u) = S13(x+bo) + pcat @ C13 (C13 = window-summed C2S2, host-made).
     The x-side scans/diffs run at kernel start, hiding the WPK DMA.
   - FFN in fp8e4 DoubleRow (weights x16 host-side; 1/16 folded into the
     gelu input scale and the final residual add).
   - bo folded into the host-prepared x (XTB = (x+bo)^T).

  Layout: device works feature-major ([D, Lpad]) in bf16; token-major fp8
  copy (XBF) only for the DFT.
"""

import numpy as np

B, L, D, H, M, DFF = 16, 2048, 512, 8, 64, 2048
E = D // H
NC_ = 8
BLOC = B // NC_          # batches per core
MEXT = 2 * M             # re|im rows
NDC = D // 128           # 4 feature tiles
NFF = DFF // 128         # 16 dff tiles
NLC = L // 128           # 16 token chunks of 128
NTC = L // 512           # 4 token chunks of 512
PADL = 13                # left replicate pad (cumsum needs one extra)
PADR = 12
LP = 2080                # PADL + L + PADR + 7 spare zeros
D0 = PADL                # data column offset in padded tiles
WPKSH = float(2 ** 17)   # fp8 scale for Fourier weights
FFNS = 16.0              # fp8 scale for FFN weights

_prog_cache = {}
_fixn = [0]


def _fix_sync_waits(nc, max_waits=1, max_updates=4):
    """Split >max sem-waits/updates per instruction onto adjacent nops.

    The AWS neuronx-cc walrus rejects instructions carrying too many sync
    commands ("Too many sync wait commands"); Tile's tail drain aggregates one
    wait per outstanding semaphore. Engine-order execution makes the split
    semantically identical.
    """
    import concourse.mybir as mybir

    for f in nc.m.functions:
        for bb in f.blocks:
            insts = bb.instructions
            i = 0
            while i < len(insts):
                ins = insts[i]
                si = ins.sync_info
                if si is not None and si.on_wait and len(si.on_wait) > max_waits:
                    waits = list(si.on_wait)
                    si.on_wait = waits[-max_waits:]
                    rest = waits[:-max_waits]
                    chunks = [rest[j:j + max_waits]
                              for j in range(0, len(rest), max_waits)]
                    for c in reversed(chunks):
                        _fixn[0] += 1
                        nop = mybir.InstNoOp(name=f"I-fixw-{_fixn[0]}", ins=[], outs=[])
                        nop.engine = ins.engine
                        nop.sync_info = mybir.SyncInfo(on_wait=c, on_update=[])
                        insts.insert(i, nop)
                        i += 1
                if si is not None and si.on_update and len(si.on_update) > max_updates:
                    ups = list(si.on_update)
                    si.on_update = ups[:max_updates]
                    rest = ups[max_updates:]
                    chunks = [rest[j:j + max_updates]
                              for j in range(0, len(rest), max_updates)]
                    for c in chunks:
                        _fixn[0] += 1
                        nop = mybir.InstNoOp(name=f"I-fixu-{_fixn[0]}", ins=[], outs=[])
                        nop.engine = ins.engine
                        nop.sync_info = mybir.SyncInfo(on_wait=[], on_update=c)
                        insts.insert(i + 1, nop)
                        i += 1
                i += 1


def _build_program(need_bq, j0, fix=True):
    import concourse.bass as bass
    import concourse.mybir as mybir
    from concourse.tile import TileContext

    F32 = mybir.dt.float32
    BF16 = mybir.dt.bfloat16
    FP8 = mybir.dt.float8e4
    AF = mybir.ActivationFunctionType
    OP = mybir.AluOpType
    DR = mybir.MatmulPerfMode.DoubleRow

    nc = bass.Bass()

    # ---- DRAM I/O ----
    XTB = nc.dram_tensor("XTB", [BLOC, D, LP], BF16, kind="ExternalInput")
    XBF = nc.dram_tensor("XBF", [BLOC, 128, NLC * D], FP8, kind="ExternalInput")
    FCT = nc.dram_tensor("FCT", [128, NLC * 128], FP8, kind="ExternalInput")
    C2S2 = nc.dram_tensor("C2S2", [128, L], BF16, kind="ExternalInput")
    C13 = nc.dram_tensor("C13", [128, L], BF16, kind="ExternalInput")
    C25 = nc.dram_tensor("C25", [128, L], BF16, kind="ExternalInput")
    WQT = nc.dram_tensor("WQT", [D, D], BF16, kind="ExternalInput")
    WOT = nc.dram_tensor("WOT", [D, D], BF16, kind="ExternalInput")
    WPK = nc.dram_tensor("WPK", [H, 128, M * 128], FP8, kind="ExternalInput")
    W1T = nc.dram_tensor("W1T", [128, NDC, DFF], FP8, kind="ExternalInput")
    W2T = nc.dram_tensor("W2T", [128, NFF, D], FP8, kind="ExternalInput")
    EYE = nc.dram_tensor("EYE", [128, 128], BF16, kind="ExternalInput")
    BQ4 = nc.dram_tensor("BQ4", [128, NDC], F32, kind="ExternalInput")
    DECS = nc.dram_tensor("DECS", [128, 4], F32, kind="ExternalInput")
    F16 = mybir.dt.float16
    OUT_T = nc.dram_tensor("OUT_T", [BLOC, D, L], F16, kind="ExternalOutput")

    with TileContext(nc) as tc:
        # ---------- persistent pools (LIFO: wpkp/fr close after fourier,
        # ffnw after the FFN, the rest at the end) ----------
        cst_cm = tc.tile_pool(name="cst", bufs=1)
        cst = cst_cm.__enter__()
        main_cm = tc.tile_pool(name="main", bufs=1)
        mainp = main_cm.__enter__()
        ear_cm = tc.tile_pool(name="ear", bufs=1)
        ear = ear_cm.__enter__()
        ffnw_cm = tc.tile_pool(name="ffnw", bufs=1)
        ffnw = ffnw_cm.__enter__()
        fr_cm = tc.tile_pool(name="fr", bufs=1)
        fr = fr_cm.__enter__()

        # DFT inputs first (DFT is the head of the dependency chain), then the
        # first WPK chunks (mode-mix stream), then x, then later-used consts.
        fct = cst.tile([128, NLC * 128], FP8, name="fct")
        nc.sync.dma_start(out=fct[:], in_=FCT[:])
        xbfs = [cst.tile([128, NLC * D], FP8, name=f"xbf{b}", tag=f"xbf{b}")
                for b in range(BLOC)]
        for b in range(BLOC):
            for s in range(4):
                nc.sync.dma_start(out=xbfs[b][:, s * 2048:(s + 1) * 2048],
                                  in_=XBF[b][:, s * 2048:(s + 1) * 2048])
        wqt = [cst.tile([128, D], BF16, name=f"wqt{i}") for i in range(NDC)]
        for i in range(NDC):
            nc.sync.dma_start(out=wqt[i][:], in_=WQT[i * 128:(i + 1) * 128, :])

        wpk_cm = tc.tile_pool(name="wpkp", bufs=4)
        wpkp = wpk_cm.__enter__()

        def wpk_dma(t_, h, q):
            # 4-way split -> 4 DMA queues per chunk (single-queue DMA is
            # ~17 GB/s; the 8.4 MB WPK stream needs many queues in flight)
            for s in range(4):
                nc.sync.dma_start(
                    out=t_[:, s * 512:(s + 1) * 512],
                    in_=WPK[h][:, q * 2048 + s * 512: q * 2048 + (s + 1) * 512])

        wpk_pre = []
        for q in range(4):  # first head's chunks, prefetched from t=0
            t_ = wpkp.tile([128, 16 * 128], FP8, name=f"wpk0_{q}", tag="wpk")
            wpk_dma(t_, 0, q)
            wpk_pre.append(t_)

        # main activation tiles: (x+bo) -> u -> r1 -> v, in place, bf16 padded
        mt = [[mainp.tile([128, LP], BF16, name=f"m_{b}_{dc}")
               for dc in range(NDC)] for b in range(BLOC)]
        for b in range(BLOC):
            for dc in range(NDC):
                nc.sync.dma_start(out=mt[b][dc][:],
                                  in_=XTB[b, dc * 128:(dc + 1) * 128, :])

        c2s2 = cst.tile([128, L], BF16, name="c2s2")
        c13 = cst.tile([128, L], BF16, name="c13")
        c25 = cst.tile([128, L], BF16, name="c25")
        nc.sync.dma_start(out=c2s2[:], in_=C2S2[:])
        nc.sync.dma_start(out=c13[:], in_=C13[:])
        nc.sync.dma_start(out=c25[:], in_=C25[:])
        wot = [cst.tile([128, D], BF16, name=f"wot{i}") for i in range(NDC)]
        for i in range(NDC):
            nc.sync.dma_start(out=wot[i][:], in_=WOT[i * 128:(i + 1) * 128, :])
        eye = cst.tile([128, 128], BF16, name="eye")
        nc.sync.dma_start(out=eye[:], in_=EYE[:])
        decs = cst.tile([128, 4], F32, name="decs")
        nc.sync.dma_start(out=decs[:], in_=DECS[:])
        ones13 = cst.tile([128, PADL], BF16, name="ones13")
        nc.vector.memset(ones13[:], 1.0)
        bq4 = None
        if need_bq:
            bq4 = cst.tile([128, NDC], F32, name="bq4")
            nc.sync.dma_start(out=bq4[:], in_=BQ4[:])

        # early pool tiles: rotating cumsums + per-tile windowed sums
        NCS = 2
        def cs_tile(i):
            return ear.tile([128, LP], F32, name="cs", tag=f"cs{i % NCS}")
        def stage_tile(i):
            return ear.tile([128, L], F16, name="stg", tag=f"stg{i % 2}")
        def scr_tile(i):
            return ear.tile([128, L], F32, name="scr", tag="scr0")
        m13x = [[ear.tile([128, L], BF16, name=f"m13x{b}{dc}", tag=f"m13x{b}{dc}")
                 for dc in range(NDC)] for b in range(BLOC)]
        m25x = [[ear.tile([128, L], BF16, name=f"m25x{b}{dc}", tag=f"m25x{b}{dc}")
                 for dc in range(NDC)] for b in range(BLOC)]

        # ---------- early: scans + windowed diffs of (x+bo), fills the ----
        # ---------- window where the PE waits on the WPK weight stream ----
        csi = 0
        for b in range(BLOC):
            for dc in range(NDC):
                cs = cs_tile(csi)
                scr = scr_tile(csi)
                csi += 1
                u = mt[b][dc]
                nc.vector.tensor_tensor_scan(cs[:], u[:], u[:], 0.0,
                                             OP.add, OP.bypass)
                # S13(t) = cs[t+19] - cs[t+6]; S25(t) = cs[t+25] - cs[t]
                nc.vector.tensor_tensor(m13x[b][dc][:], cs[:, 19:2067],
                                        cs[:, 6:2054], OP.subtract)
                nc.scalar.mul(m13x[b][dc][:], m13x[b][dc][:], 1.0 / 13.0)
                nc.gpsimd.tensor_tensor(scr[:], cs[:, 25:2073],
                                        cs[:, 0:2048], OP.subtract)
                nc.scalar.mul(m25x[b][dc][:], scr[:], 1.0 / 25.0)

        # ---------- Fourier branch (fp8/bf16) ----------
        with tc.tile_pool(name="frp", bufs=2, space="PSUM") as frp:
            qt = [[None] * NDC for _ in range(BLOC)]
            for b in range(BLOC):
                xbf = xbfs[b]
                # DFT: xselT[d, m-ext] = sum_l x[l, d] * Fcat[l, m-ext]
                xselT = fr.tile([128, NDC * 128], BF16, name=f"xselT{b}",
                                tag=f"xselT{b}")
                for dc in range(NDC):
                    ps = frp.tile([128, 128], F32, name="psA", tag="psA")
                    for lc in range(NLC):
                        nc.tensor.matmul(
                            ps[:],
                            xbf[:, lc * D + dc * 128: lc * D + (dc + 1) * 128],
                            fct[:, lc * 128:(lc + 1) * 128],
                            start=(lc == 0), stop=(lc == NLC - 1))
                    nc.scalar.copy(xselT[:, dc * 128:(dc + 1) * 128], ps[:])
                # q-projection in mode space (fp8 out for the mode mix)
                for do in range(NDC):
                    qt[b][do] = fr.tile([128, 128], FP8, name=f"qt{b}_{do}",
                                        tag=f"qt{b}_{do}")
                    ps = frp.tile([128, 128], F32, name="psQ", tag="psA")
                    for dc in range(NDC):
                        nc.tensor.matmul(
                            ps[:], wqt[dc][:, do * 128:(do + 1) * 128],
                            xselT[:, dc * 128:(dc + 1) * 128],
                            start=(dc == 0), stop=(dc == NDC - 1))
                    if need_bq:
                        nc.vector.tensor_tensor(
                            ps[:, j0:j0 + 1], ps[:, j0:j0 + 1],
                            bq4[:, do:do + 1], OP.add)
                    nc.scalar.copy(qt[b][do][:], ps[:])

            # mode mix: RH_h rows 0:64 = Qre, 64:128 = Qim; col = 2m + b
            rh = [fr.tile([128, 128], FP8, name=f"rh{h}", tag=f"rh{h}")
                  for h in range(H)]
            for h in range(H):
                src_do, r0 = h // 2, (h % 2) * 64
                for b in range(BLOC):
                    rhv = rh[h].rearrange("p (m t) -> p m t", t=2)
                    nc.scalar.copy(rhv[0:64, :, b], qt[b][src_do][r0:r0 + 64, 0:64])
                    nc.scalar.copy(rhv[64:128, :, b], qt[b][src_do][r0:r0 + 64, 64:128])
            otre = [[fr.tile([128, M], BF16, name=f"otre{b}_{dc}", tag=f"otre{b}{dc}")
                     for dc in range(NDC)] for b in range(BLOC)]
            otim = [[fr.tile([128, M], BF16, name=f"otim{b}_{dc}", tag=f"otim{b}{dc}")
                     for dc in range(NDC)] for b in range(BLOC)]
            for h in range(H):
                psm = frp.tile([128, 128], F32, name="psM", tag="psM")
                for q in range(4):
                    if h == 0:
                        wpk_q = wpk_pre[q]
                    else:
                        wpk_q = wpkp.tile([128, 16 * 128], FP8,
                                          name=f"wpk{h}_{q}", tag="wpk")
                        wpk_dma(wpk_q, h, q)
                    for mq in range(16):
                        m = q * 16 + mq
                        nc.tensor.matmul(
                            psm[:, 2 * m:2 * m + 2],
                            wpk_q[:, mq * 128:(mq + 1) * 128],
                            rh[h][:, 2 * m:2 * m + 2],
                            start=True, stop=True)
                psv = psm.rearrange("p (m t) -> p m t", t=2)
                dc, r0 = h // 2, (h % 2) * 64
                for b in range(BLOC):
                    nc.scalar.copy(otre[b][dc][r0:r0 + 64, :], psv[0:64, :, b])
                    nc.scalar.copy(otim[b][dc][r0:r0 + 64, :], psv[64:128, :, b])

            # Wo projection in mode space, then transpose into pcat_b
            pcat = [fr.tile([128, D], BF16, name=f"pcat{b}", tag=f"pcat{b}")
                    for b in range(BLOC)]
            for b in range(BLOC):
                for ro, ot in ((0, otre[b]), (64, otim[b])):
                    for do in range(NDC):
                        ps = frp.tile([128, M], F32, name="psP", tag="psP")
                        for dc in range(NDC):
                            nc.tensor.matmul(
                                ps[:], wot[dc][:, do * 128:(do + 1) * 128],
                                ot[dc][:], start=(dc == 0), stop=(dc == NDC - 1))
                        pp = fr.tile([128, M], BF16, name=f"pp{ro}_{do}", tag="pp")
                        nc.scalar.copy(pp[:], ps[:])
                        pst = frp.tile([M, 128], BF16, name="psT", tag="psT")
                        nc.tensor.transpose(pst[:], pp[:], eye[:])
                        nc.scalar.copy(pcat[b][ro:ro + 64, do * 128:(do + 1) * 128],
                                       pst[:])

        # FFN weights arrive while decomp1 runs
        w1t = ffnw.tile([128, NDC, DFF], FP8, name="w1t")
        for s in range(4):
            nc.sync.dma_start(out=w1t[:, s, :], in_=W1T[:, s, :])
        w2t = ffnw.tile([128, NFF, D], FP8, name="w2t")
        for s in range(4):
            nc.sync.dma_start(out=w2t[:, 4 * s:4 * (s + 1), :],
                              in_=W2T[:, 4 * s:4 * (s + 1), :])

        # ---------- iDFT + decomp1 late combine ----------
        # per (b,dc,t4): psy=(y + x) ; ps13=(Y13'+m13x) ; ps25=(Y25'+m25x)
        # u = copy(psy) ; g=sig(u) ; h=1-g ; r = u - ps13*g - ps25*h
        dl_cm = tc.tile_pool(name="dl", bufs=2)
        dl = dl_cm.__enter__()
        psy_cm = tc.tile_pool(name="psy", bufs=2, space="PSUM")
        psyp = psy_cm.__enter__()
        for b in range(BLOC):
            for dc in range(NDC):
                dcb = slice(dc * 128, (dc + 1) * 128)
                for t4 in range(NTC):
                    ts_ = slice(t4 * 512, (t4 + 1) * 512)
                    mts = mt[b][dc][:, D0 + t4 * 512: D0 + (t4 + 1) * 512]
                    psy = psyp.tile([128, 512], F32, name="psy", tag="psy")
                    nc.tensor.matmul(psy[:], pcat[b][:, dcb], c2s2[:, ts_],
                                     start=True, stop=False)
                    nc.tensor.matmul(psy[:], eye[:], mts,
                                     start=False, stop=True)
                    ps13 = psyp.tile([128, 512], F32, name="ps13", tag="ps13")
                    nc.tensor.matmul(ps13[:], pcat[b][:, dcb], c13[:, ts_],
                                     start=True, stop=False)
                    nc.tensor.matmul(ps13[:], eye[:], m13x[b][dc][:, ts_],
                                     start=False, stop=True)
                    ps25 = psyp.tile([128, 512], F32, name="ps25", tag="ps25")
                    nc.tensor.matmul(ps25[:], pcat[b][:, dcb], c25[:, ts_],
                                     start=True, stop=False)
                    nc.tensor.matmul(ps25[:], eye[:], m25x[b][dc][:, ts_],
                                     start=False, stop=True)
                    # element combine: r = u - ma25 - g*(ma13 - ma25)
                    # (u = psy stays in PSUM; sigmoid/subs read it directly)
                    gt = dl.tile([128, 512], BF16, name="gt", tag="gt")
                    m2 = dl.tile([128, 512], BF16, name="m2", tag="m2")
                    dx = dl.tile([128, 512], BF16, name="dx", tag="dx")
                    ft = dl.tile([128, 512], BF16, name="ft", tag="ft")
                    nc.scalar.activation(gt[:], psy[:], AF.Sigmoid,
                                         scale=decs[:, 0:1], bias=decs[:, 1:2])
                    nc.scalar.copy(m2[:], ps25[:])                   # ma25 (bf16)
                    nc.vector.tensor_tensor(dx[:], ps13[:], m2[:], OP.subtract)
                    nc.vector.tensor_tensor(dx[:], dx[:], gt[:], OP.mult)
                    nc.vector.tensor_tensor(ft[:], psy[:], m2[:], OP.subtract)
                    nc.gpsimd.tensor_tensor(mts, ft[:], dx[:], OP.subtract)
        psy_cm.__exit__(None, None, None)
        dl_cm.__exit__(None, None, None)
        wpk_cm.__exit__(None, None, None)
        fr_cm.__exit__(None, None, None)

        # ---------- FFN (fp8 DoubleRow) + decomp2 ----------
        # Engine plan: FFN(b0) element ops on DVE; decomp2(b0) on DVE+ACT
        # (issued between the two FFN batches, overlapping FFN(b1) on PE);
        # FFN(b1) element ops on ACT+Pool; decomp2(b1) split DVE+Pool.
        def pass2(b, dc, sidx):
            """v (mt, padded bf16) -> series-decomp residual -> f16 stage -> DMA."""
            u = mt[b][dc]
            # refresh replicate pads from v (fp32 edge columns for the scalar op)
            ec = ear.tile([128, 2], F32, name="ec", tag=f"ec{sidx % 2}")
            nc.vector.tensor_copy(ec[:, 0:1], u[:, D0:D0 + 1])
            nc.vector.tensor_copy(ec[:, 1:2], u[:, D0 + L - 1:D0 + L])
            nc.vector.tensor_scalar_mul(u[:, 0:D0], ones13[:], ec[:, 0:1])
            nc.vector.tensor_scalar_mul(u[:, D0 + L:D0 + L + PADR],
                                        ones13[:, 0:PADR], ec[:, 1:2])
            cs = cs_tile(sidx)
            nc.vector.tensor_tensor_scan(cs[:], u[:], u[:], 0.0, OP.add, OP.bypass)
            # reuse m-tile storage of this (b,dc) + the sibling batch's tiles
            d13 = ear.tile([128, L], BF16, name="d13", tag=f"m13x{b}{dc}")
            m25 = ear.tile([128, L], BF16, name="m25", tag=f"m25x{b}{dc}")
            ob = 1 - b
            gt = ear.tile([128, L], BF16, name="gt2", tag=f"m13x{ob}{dc}")
            ft = ear.tile([128, L], BF16, name="ft2", tag=f"m25x{ob}{dc}")
            scr = scr_tile(sidx)
            nc.vector.tensor_tensor(d13[:], cs[:, 19:2067], cs[:, 6:2054],
                                    OP.subtract)
            nc.gpsimd.tensor_tensor(scr[:], cs[:, 25:2073], cs[:, 0:2048],
                                    OP.subtract)
            nc.scalar.mul(m25[:], scr[:], 1.0 / 25.0)
            ud = u[:, D0:D0 + L]
            nc.scalar.activation(gt[:], ud, AF.Sigmoid,
                                 scale=decs[:, 2:3], bias=decs[:, 3:4])
            # r = v - m25 - g*(m13 - m25)
            nc.vector.tensor_scalar_mul(d13[:], d13[:], 1.0 / 13.0)
            nc.vector.tensor_tensor(d13[:], d13[:], m25[:], OP.subtract)
            nc.vector.tensor_tensor(d13[:], d13[:], gt[:], OP.mult)
            nc.gpsimd.tensor_tensor(ft[:], ud, m25[:], OP.subtract)
            stg = stage_tile(sidx)
            nc.vector.tensor_tensor(stg[:], ft[:], d13[:], OP.subtract)
            nc.sync.dma_start(out=OUT_T[b, dc * 128:(dc + 1) * 128, :],
                              in_=stg[:])

        with tc.tile_pool(name="ffa", bufs=2) as ffa, \
             tc.tile_pool(name="ffb", bufs=1) as ffb, \
             tc.tile_pool(name="gqp", bufs=1) as gqp, \
             tc.tile_pool(name="tmq", bufs=2) as tmq, \
             tc.tile_pool(name="pshp", bufs=3, space="PSUM") as pshp, \
             tc.tile_pool(name="psfp", bufs=1, space="PSUM") as psfp:
            r1b1 = [ffb.tile([128, NDC, 512], FP8, name=f"r1b1_{t4}",
                             tag=f"r1b1_{t4}") for t4 in range(NTC)]

            def ffn_t4(b, t4):
                if b == 0:
                    r1c = ffa.tile([128, NDC, 512], FP8, name="r1c", tag="r1c")
                    for dc in range(NDC):
                        src = mt[b][dc][:, D0 + t4 * 512: D0 + (t4 + 1) * 512]
                        nc.vector.tensor_copy(r1c[:, dc, :], src)
                else:
                    r1c = r1b1[t4]
                gq = gqp.tile([128, NFF, 512], FP8, name="gq", tag="gq")
                for ff in range(NFF):
                    psh = pshp.tile([128, 512], F32, name="psH", tag="psH")
                    nc.tensor.matmul(psh[:], w1t[:, 0:2, ff * 128:(ff + 1) * 128],
                                     r1c[:, 0:2, :], start=True, stop=False,
                                     perf_mode=DR)
                    nc.tensor.matmul(psh[:], w1t[:, 2:4, ff * 128:(ff + 1) * 128],
                                     r1c[:, 2:4, :], start=False, stop=True,
                                     perf_mode=DR)
                    nc.scalar.activation(gq[:, ff, :], psh[:], AF.Gelu,
                                         scale=1.0 / FFNS)
                psf = [psfp.tile([128, 512], F32, name=f"psF{do}", tag=f"psF{do}")
                       for do in range(NDC)]
                for do in range(NDC):
                    for sp in range(0, NFF, 2):
                        nc.tensor.matmul(
                            psf[do][:], w2t[:, sp:sp + 2, do * 128:(do + 1) * 128],
                            gq[:, sp:sp + 2, :], start=(sp == 0),
                            stop=(sp == NFF - 2), perf_mode=DR)
                for do in range(NDC):
                    sl = mt[b][do][:, D0 + t4 * 512: D0 + (t4 + 1) * 512]
                    if b == 0:
                        nc.vector.scalar_tensor_tensor(
                            sl, psf[do][:], 1.0 / FFNS, sl, OP.mult, OP.add)
                    else:
                        # ACT drains psf, Pool adds (keeps DVE on decomp2(b0))
                        tm = tmq.tile([128, 512], BF16, name="tm", tag="tm")
                        nc.scalar.mul(tm[:], psf[do][:], 1.0 / FFNS)
                        nc.gpsimd.tensor_tensor(sl, sl, tm[:], OP.add)

            for t4 in range(NTC):
                ffn_t4(0, t4)
            # batch-1 FFN inputs cast on ACT (keeps DVE free for decomp2(b0))
            for t4 in range(NTC):
                for dc in range(NDC):
                    src = mt[1][dc][:, D0 + t4 * 512: D0 + (t4 + 1) * 512]
                    nc.scalar.copy(r1b1[t4][:, dc, :], src)
            # interleave: FFN(b1) on PE/ACT/Pool while decomp2(b0) runs on DVE
            for t4 in range(NTC):
                ffn_t4(1, t4)
                pass2(0, t4, t4)
            for dc in range(NDC):
                pass2(1, dc, NDC + dc)

        ffnw_cm.__exit__(None, None, None)
        ear_cm.__exit__(None, None, None)
        main_cm.__exit__(None, None, None)
        cst_cm.__exit__(None, None, None)

    if fix:
        _fix_sync_waits(nc)
    return nc


def _host_prep(inputs):
    import ml_dtypes
    bf16 = ml_dtypes.bfloat16
    fp8 = ml_dtypes.float8_e4m3
    x = np.asarray(inputs["x"], np.float32)
    bo = np.asarray(inputs["bo"], np.float32)
    modes = np.asarray(inputs["mode_index"]).astype(np.int64)
    l = np.arange(L, dtype=np.float64)
    ang = 2.0 * np.pi * np.outer(l, modes.astype(np.float64)) / L
    FC = np.concatenate([np.cos(ang), -np.sin(ang)], axis=1)          # [L, 128]
    m_out = np.arange(M, dtype=np.float64)
    w = np.where(m_out == 0, 1.0, 2.0) / L
    ang2 = 2.0 * np.pi * np.outer(m_out, l) / L
    C2 = np.concatenate([w[:, None] * np.cos(ang2),
                         w[:, None] * -np.sin(ang2)], axis=0)         # [128, L]
    C2 = C2 / WPKSH                                      # fp8 WPK compensation
    # replicate-clamped window sums of C2 (the y-side of decomp1 split)
    idx = np.arange(L)
    C13w = np.zeros_like(C2)
    for j in range(-6, 7):
        C13w += C2[:, np.clip(idx + j, 0, L - 1)]
    C13w /= 13.0
    C25w = np.zeros_like(C2)
    for j in range(-12, 13):
        C25w += C2[:, np.clip(idx + j, 0, L - 1)]
    C25w /= 25.0

    FCT = FC.reshape(NLC, 128, 128).transpose(1, 0, 2).reshape(128, NLC * 128)

    wr = np.asarray(inputs["four_wr"], np.float64)   # [H, E, O, M]
    wi = np.asarray(inputs["four_wi"], np.float64)
    wpk = np.zeros((H, M, 128, 128), np.float64)
    wpk[:, :, 0:64, 0:64] = wr.transpose(0, 3, 1, 2)
    wpk[:, :, 0:64, 64:128] = wi.transpose(0, 3, 1, 2)
    wpk[:, :, 64:128, 0:64] = -wi.transpose(0, 3, 1, 2)
    wpk[:, :, 64:128, 64:128] = wr.transpose(0, 3, 1, 2)
    WPKh = (wpk.transpose(0, 2, 1, 3).reshape(H, 128, M * 128)) * WPKSH

    dec1_w = np.asarray(inputs["dec1_w"], np.float64)
    dec1_b = np.asarray(inputs["dec1_b"], np.float64)
    dec2_w = np.asarray(inputs["dec2_w"], np.float64)
    dec2_b = np.asarray(inputs["dec2_b"], np.float64)
    decs = np.zeros((128, 4), np.float32)
    decs[:, 0] = dec1_w[0] - dec1_w[1]
    decs[:, 1] = dec1_b[0] - dec1_b[1]
    decs[:, 2] = dec2_w[0] - dec2_w[1]
    decs[:, 3] = dec2_b[0] - dec2_b[1]

    bq = np.asarray(inputs["bq"], np.float32)
    zero_pos = np.nonzero(modes == 0)[0]
    need_bq = bool(len(zero_pos)) and bool(np.any(bq != 0))
    j0 = int(zero_pos[0]) if need_bq else 0
    BQ4 = np.ascontiguousarray((L * bq).reshape(NDC, 128).T).astype(np.float32)

    # FFN weights: [128, S, F] fp8 with k-subtile interleave, x16
    w1 = np.asarray(inputs["conv1_w"], np.float32)   # [DFF, D]
    w2 = np.asarray(inputs["conv2_w"], np.float32)   # [D, DFF]
    W1T = (w1.T.reshape(NDC, 128, DFF) * FFNS).astype(fp8)          # [s,p,f]
    W1T = np.ascontiguousarray(W1T.transpose(1, 0, 2))              # [128,s,f]
    W2T = (w2.T.reshape(NFF, 128, D) * FFNS).astype(fp8)
    W2T = np.ascontiguousarray(W2T.transpose(1, 0, 2))

    shared = {
        "FCT": FCT.astype(fp8),
        "C2S2": C2.astype(bf16),
        "C13": C13w.astype(bf16),
        "C25": C25w.astype(bf16),
        "WQT": np.ascontiguousarray(np.asarray(inputs["Wq"], np.float32).T).astype(bf16),
        "WOT": np.ascontiguousarray(np.asarray(inputs["Wo"], np.float32).T).astype(bf16),
        "WPK": WPKh.astype(fp8),
        "W1T": W1T, "W2T": W2T,
        "EYE": np.eye(128, dtype=np.float32).astype(bf16),
        "BQ4": BQ4, "DECS": decs,
    }
    in_maps = []
    for c in range(NC_):
        xl = x[c * BLOC:(c + 1) * BLOC]                       # [2, L, D]
        xt = (xl + bo[None, None, :]).transpose(0, 2, 1)      # [2, D, L]
        xtp = np.zeros((BLOC, D, LP), np.float32)
        xtp[:, :, D0:D0 + L] = xt
        xtp[:, :, 0:D0] = xt[:, :, 0:1]
        xtp[:, :, D0 + L:D0 + L + PADR] = xt[:, :, L - 1:L]
        xbf = xl.astype(fp8)                                  # [2, L, D]
        XBFc = np.ascontiguousarray(
            xbf.reshape(BLOC, NLC, 128, D).transpose(0, 2, 1, 3)
        ).reshape(BLOC, 128, NLC * D)
        im = dict(shared)
        im["XTB"] = xtp.astype(bf16)
        im["XBF"] = XBFc
        in_maps.append(im)
    return in_maps, need_bq, j0


def kernel(**inputs):
    from concourse.bass_utils import run_bass_kernel_spmd

    in_maps, need_bq, j0 = _host_prep(inputs)
    key = (need_bq, j0)
    if key not in _prog_cache:
        _prog_cache[key] = _build_program(need_bq, j0)
    nc = _prog_cache[key]
    res = run_bass_kernel_spmd(nc, in_maps, core_ids=list(range(NC_)))
    outs = []
    for c in range(NC_):
        ot = np.asarray(res.results[c]["OUT_T"])              # [2, D, L]
        outs.append(np.ascontiguousarray(ot.transpose(0, 2, 1)))
    return np.concatenate(outs, axis=0).astype(np.float32)


# revision 47
# speedup vs baseline: 1.2488x; 1.0733x over previous
"""FEDformer encoder layer on 8 TRN2 NeuronCores — batch-data-parallel Bass kernel.

Strategy (self-contained; shapes hardcoded):
  B=16,L=2048,D=512,H=8,E=64,M=64,DFF=2048; 8 cores x 2 batches each; no collectives.

  Math restructuring (validated against the jax reference):
   - rfft+mode-gather == x @ Fcat where Fcat[l, 0:64]=cos(2*pi*k_j*l/L),
     Fcat[l, 64:128]=-sin(...), k_j = mode_index.
   - Wq/Wo commute with the DFT -> applied in mode space. k/v projections are
     dead code in the reference.
   - irfft of a spectrum with only the selected modes == P @ C2S2.
   - The Fourier branch contributes ~1e-5 absolute to an O(1) output, so the
     whole branch runs in fp8/bf16 (WPK pre-scaled by 2^17 on host; 2^-17
     folded into the iDFT matrices).
   - series-decomp: K=2 softmax == sigmoid of weight/bias deltas; moving
     averages via fp32 cumsum over a replicate-padded tile + shifted
     subtracts (pads baked into the padded layout; no edge fixups).
   - decomp1 split trick: u = (x+bo) + y with y = pcat @ C2S2 linear, so
     S13(u) = S13(x+bo) + pcat @ C13 (C13 = window-summed C2S2, host-made).
     The x-side scans/diffs run at kernel start, hiding the WPK DMA.
   - FFN in fp8e4 DoubleRow (weights x16 host-side; 1/16 folded into the
     gelu input scale and the final residual add).
   - bo folded into the host-prepared x (XTB = (x+bo)^T).

  Layout: device works feature-major ([D, Lpad]) in bf16; token-major fp8
  copy (XBF) only for the DFT.
"""

import numpy as np

B, L, D, H, M, DFF = 16, 2048, 512, 8, 64, 2048
E = D // H
NC_ = 8
BLOC = B // NC_          # batches per core
MEXT = 2 * M             # re|im rows
NDC = D // 128           # 4 feature tiles
NFF = DFF // 128         # 16 dff tiles
NLC = L // 128           # 16 token chunks of 128
NTC = L // 512           # 4 token chunks of 512
PADL = 13                # left replicate pad (cumsum needs one extra)
PADR = 12
LP = 2080                # PADL + L + PADR + 7 spare zeros
D0 = PADL                # data column offset in padded tiles
WPKSH = float(2 ** 17)   # fp8 scale for Fourier weights
FFNS = 16.0              # fp8 scale for FFN weights

_prog_cache = {}
_fixn = [0]


def _fix_sync_waits(nc, max_waits=1, max_updates=4):
    """Split >max sem-waits/updates per instruction onto adjacent nops.

    The AWS neuronx-cc walrus rejects instructions carrying too many sync
    commands ("Too many sync wait commands"); Tile's tail drain aggregates one
    wait per outstanding semaphore. Engine-order execution makes the split
    semantically identical.
    """
    import concourse.mybir as mybir

    for f in nc.m.functions:
        for bb in f.blocks:
            insts = bb.instructions
            i = 0
            while i < len(insts):
                ins = insts[i]
                si = ins.sync_info
                if si is not None and si.on_wait and len(si.on_wait) > max_waits:
                    waits = list(si.on_wait)
                    si.on_wait = waits[-max_waits:]
                    rest = waits[:-max_waits]
                    chunks = [rest[j:j + max_waits]
                              for j in range(0, len(rest), max_waits)]
                    for c in reversed(chunks):
                        _fixn[0] += 1
                        nop = mybir.InstNoOp(name=f"I-fixw-{_fixn[0]}", ins=[], outs=[])
                        nop.engine = ins.engine
                        nop.sync_info = mybir.SyncInfo(on_wait=c, on_update=[])
                        insts.insert(i, nop)
                        i += 1
                if si is not None and si.on_update and len(si.on_update) > max_updates:
                    ups = list(si.on_update)
                    si.on_update = ups[:max_updates]
                    rest = ups[max_updates:]
                    chunks = [rest[j:j + max_updates]
                              for j in range(0, len(rest), max_updates)]
                    for c in chunks:
                        _fixn[0] += 1
                        nop = mybir.InstNoOp(name=f"I-fixu-{_fixn[0]}", ins=[], outs=[])
                        nop.engine = ins.engine
                        nop.sync_info = mybir.SyncInfo(on_wait=[], on_update=c)
                        insts.insert(i + 1, nop)
                        i += 1
                i += 1


def _build_program(need_bq, j0, fix=True):
    import concourse.bass as bass
    import concourse.mybir as mybir
    from concourse.tile import TileContext

    F32 = mybir.dt.float32
    BF16 = mybir.dt.bfloat16
    FP8 = mybir.dt.float8e4
    AF = mybir.ActivationFunctionType
    OP = mybir.AluOpType
    DR = mybir.MatmulPerfMode.DoubleRow

    nc = bass.Bass()

    # ---- DRAM I/O ----
    XTB = nc.dram_tensor("XTB", [BLOC, D, LP], BF16, kind="ExternalInput")
    XBF = nc.dram_tensor("XBF", [BLOC, 128, NLC * D], FP8, kind="ExternalInput")
    FCT = nc.dram_tensor("FCT", [128, NLC * 128], FP8, kind="ExternalInput")
    C2S2 = nc.dram_tensor("C2S2", [128, L], BF16, kind="ExternalInput")
    C13 = nc.dram_tensor("C13", [128, L], BF16, kind="ExternalInput")
    C25 = nc.dram_tensor("C25", [128, L], BF16, kind="ExternalInput")
    WQT = nc.dram_tensor("WQT", [D, D], BF16, kind="ExternalInput")
    WOT = nc.dram_tensor("WOT", [D, D], BF16, kind="ExternalInput")
    WPK = nc.dram_tensor("WPK", [H, 128, M * 128], FP8, kind="ExternalInput")
    W1T = nc.dram_tensor("W1T", [128, NDC, DFF], FP8, kind="ExternalInput")
    W2T = nc.dram_tensor("W2T", [128, NFF, D], FP8, kind="ExternalInput")
    EYE = nc.dram_tensor("EYE", [128, 128], BF16, kind="ExternalInput")
    BQ4 = nc.dram_tensor("BQ4", [128, NDC], F32, kind="ExternalInput")
    DECS = nc.dram_tensor("DECS", [128, 4], F32, kind="ExternalInput")
    F16 = mybir.dt.float16
    OUT_T = nc.dram_tensor("OUT_T", [BLOC, D, L], F16, kind="ExternalOutput")

    with TileContext(nc) as tc:
        # ---------- persistent pools (LIFO: wpkp/fr close after fourier,
        # ffnw after the FFN, the rest at the end) ----------
        cst_cm = tc.tile_pool(name="cst", bufs=1)
        cst = cst_cm.__enter__()
        main_cm = tc.tile_pool(name="main", bufs=1)
        mainp = main_cm.__enter__()
        ear_cm = tc.tile_pool(name="ear", bufs=1)
        ear = ear_cm.__enter__()
        ffnw_cm = tc.tile_pool(name="ffnw", bufs=1)
        ffnw = ffnw_cm.__enter__()
        fr_cm = tc.tile_pool(name="fr", bufs=1)
        fr = fr_cm.__enter__()

        # DFT inputs first (DFT is the head of the dependency chain), then the
        # first WPK chunks (mode-mix stream), then x, then later-used consts.
        fct = cst.tile([128, NLC * 128], FP8, name="fct")
        nc.sync.dma_start(out=fct[:], in_=FCT[:])
        xbfs = [cst.tile([128, NLC * D], FP8, name=f"xbf{b}", tag=f"xbf{b}")
                for b in range(BLOC)]
        for b in range(BLOC):
            nc.sync.dma_start(out=xbfs[b][:], in_=XBF[b])
        wqt = [cst.tile([128, D], BF16, name=f"wqt{i}") for i in range(NDC)]
        for i in range(NDC):
            nc.sync.dma_start(out=wqt[i][:], in_=WQT[i * 128:(i + 1) * 128, :])

        # WPK stream: 2 half-head chunks per head (descriptor generation is
        # ~1us serial per dma_start, so few big DMAs beat many small ones)
        wpk_cm = tc.tile_pool(name="wpkp", bufs=3)
        wpkp = wpk_cm.__enter__()

        def wpk_chunk(h, hf):
            t_ = wpkp.tile([128, 32 * 128], FP8, name=f"wpk{h}_{hf}", tag="wpk")
            nc.sync.dma_start(out=t_[:],
                              in_=WPK[h][:, hf * 4096:(hf + 1) * 4096])
            return t_

        wpk_pre = [wpk_chunk(0, 0), wpk_chunk(0, 1), wpk_chunk(1, 0)]

        # main activation tiles: (x+bo) -> u -> r1 -> v, in place, bf16 padded
        mt = [[mainp.tile([128, LP], BF16, name=f"m_{b}_{dc}")
               for dc in range(NDC)] for b in range(BLOC)]
        for b in range(BLOC):
            for dc in range(NDC):
                nc.sync.dma_start(out=mt[b][dc][:],
                                  in_=XTB[b, dc * 128:(dc + 1) * 128, :])

        c2s2 = cst.tile([128, L], BF16, name="c2s2")
        c13 = cst.tile([128, L], BF16, name="c13")
        c25 = cst.tile([128, L], BF16, name="c25")
        nc.sync.dma_start(out=c2s2[:], in_=C2S2[:])
        nc.sync.dma_start(out=c13[:], in_=C13[:])
        nc.sync.dma_start(out=c25[:], in_=C25[:])
        wot = [cst.tile([128, D], BF16, name=f"wot{i}") for i in range(NDC)]
        for i in range(NDC):
            nc.sync.dma_start(out=wot[i][:], in_=WOT[i * 128:(i + 1) * 128, :])
        eye = cst.tile([128, 128], BF16, name="eye")
        nc.sync.dma_start(out=eye[:], in_=EYE[:])
        decs = cst.tile([128, 4], F32, name="decs")
        nc.sync.dma_start(out=decs[:], in_=DECS[:])
        ones13 = cst.tile([128, PADL], BF16, name="ones13")
        nc.vector.memset(ones13[:], 1.0)
        bq4 = None
        if need_bq:
            bq4 = cst.tile([128, NDC], F32, name="bq4")
            nc.sync.dma_start(out=bq4[:], in_=BQ4[:])

        # early pool tiles: rotating cumsums + per-tile windowed sums
        NCS = 2
        def cs_tile(i):
            return ear.tile([128, LP], F32, name="cs", tag=f"cs{i % NCS}")
        def stage_tile(i):
            return ear.tile([128, L], F16, name="stg", tag="stg0")
        def scr_tile(i):
            return ear.tile([128, L], F32, name="scr", tag="scr0")
        m13x = [[ear.tile([128, L], BF16, name=f"m13x{b}{dc}", tag=f"m13x{b}{dc}")
                 for dc in range(NDC)] for b in range(BLOC)]
        m25x = [[ear.tile([128, L], BF16, name=f"m25x{b}{dc}", tag=f"m25x{b}{dc}")
                 for dc in range(NDC)] for b in range(BLOC)]

        # ---------- early: scans + windowed diffs of (x+bo), fills the ----
        # ---------- window where the PE waits on the WPK weight stream ----
        csi = 0
        for b in range(BLOC):
            for dc in range(NDC):
                cs = cs_tile(csi)
                scr = scr_tile(csi)
                csi += 1
                u = mt[b][dc]
                nc.vector.tensor_tensor_scan(cs[:], u[:], u[:], 0.0,
                                             OP.add, OP.bypass)
                # S13(t) = cs[t+19] - cs[t+6]; S25(t) = cs[t+25] - cs[t]
                nc.vector.tensor_tensor(m13x[b][dc][:], cs[:, 19:2067],
                                        cs[:, 6:2054], OP.subtract)
                nc.scalar.mul(m13x[b][dc][:], m13x[b][dc][:], 1.0 / 13.0)
                nc.gpsimd.tensor_tensor(scr[:], cs[:, 25:2073],
                                        cs[:, 0:2048], OP.subtract)
                nc.scalar.mul(m25x[b][dc][:], scr[:], 1.0 / 25.0)

        # ---------- Fourier branch (fp8/bf16) ----------
        with tc.tile_pool(name="frp", bufs=2, space="PSUM") as frp:
            qt = [[None] * NDC for _ in range(BLOC)]
            for b in range(BLOC):
                xbf = xbfs[b]
                # DFT: xselT[d, m-ext] = sum_l x[l, d] * Fcat[l, m-ext]
                xselT = fr.tile([128, NDC * 128], BF16, name=f"xselT{b}",
                                tag=f"xselT{b}")
                for dc in range(NDC):
                    ps = frp.tile([128, 128], F32, name="psA", tag="psA")
                    for lc in range(NLC):
                        nc.tensor.matmul(
                            ps[:],
                            xbf[:, lc * D + dc * 128: lc * D + (dc + 1) * 128],
                            fct[:, lc * 128:(lc + 1) * 128],
                            start=(lc == 0), stop=(lc == NLC - 1))
                    nc.scalar.copy(xselT[:, dc * 128:(dc + 1) * 128], ps[:])
                # q-projection in mode space (fp8 out for the mode mix)
                for do in range(NDC):
                    qt[b][do] = fr.tile([128, 128], FP8, name=f"qt{b}_{do}",
                                        tag=f"qt{b}_{do}")
                    ps = frp.tile([128, 128], F32, name="psQ", tag="psA")
                    for dc in range(NDC):
                        nc.tensor.matmul(
                            ps[:], wqt[dc][:, do * 128:(do + 1) * 128],
                            xselT[:, dc * 128:(dc + 1) * 128],
                            start=(dc == 0), stop=(dc == NDC - 1))
                    if need_bq:
                        nc.vector.tensor_tensor(
                            ps[:, j0:j0 + 1], ps[:, j0:j0 + 1],
                            bq4[:, do:do + 1], OP.add)
                    nc.scalar.copy(qt[b][do][:], ps[:])

            # mode mix: RH_h rows 0:64 = Qre, 64:128 = Qim; col = 2m + b
            rh = [fr.tile([128, 128], FP8, name=f"rh{h}", tag=f"rh{h}")
                  for h in range(H)]
            for h in range(H):
                src_do, r0 = h // 2, (h % 2) * 64
                for b in range(BLOC):
                    rhv = rh[h].rearrange("p (m t) -> p m t", t=2)
                    nc.scalar.copy(rhv[0:64, :, b], qt[b][src_do][r0:r0 + 64, 0:64])
                    nc.scalar.copy(rhv[64:128, :, b], qt[b][src_do][r0:r0 + 64, 64:128])
            otre = [[fr.tile([128, M], BF16, name=f"otre{b}_{dc}", tag=f"otre{b}{dc}")
                     for dc in range(NDC)] for b in range(BLOC)]
            otim = [[fr.tile([128, M], BF16, name=f"otim{b}_{dc}", tag=f"otim{b}{dc}")
                     for dc in range(NDC)] for b in range(BLOC)]
            npre = len(wpk_pre)
            for h in range(H):
                psm = frp.tile([128, 128], F32, name="psM", tag="psM")
                for hf in range(2):
                    ci = 2 * h + hf
                    wpk_q = wpk_pre[ci] if ci < npre else wpk_chunk(h, hf)
                    for mq in range(32):
                        m = hf * 32 + mq
                        nc.tensor.matmul(
                            psm[:, 2 * m:2 * m + 2],
                            wpk_q[:, mq * 128:(mq + 1) * 128],
                            rh[h][:, 2 * m:2 * m + 2],
                            start=True, stop=True)
                psv = psm.rearrange("p (m t) -> p m t", t=2)
                dc, r0 = h // 2, (h % 2) * 64
                for b in range(BLOC):
                    nc.scalar.copy(otre[b][dc][r0:r0 + 64, :], psv[0:64, :, b])
                    nc.scalar.copy(otim[b][dc][r0:r0 + 64, :], psv[64:128, :, b])

            # Wo projection in mode space, then transpose into pcat_b
            pcat = [fr.tile([128, D], BF16, name=f"pcat{b}", tag=f"pcat{b}")
                    for b in range(BLOC)]
            for b in range(BLOC):
                for ro, ot in ((0, otre[b]), (64, otim[b])):
                    for do in range(NDC):
                        ps = frp.tile([128, M], F32, name="psP", tag="psP")
                        for dc in range(NDC):
                            nc.tensor.matmul(
                                ps[:], wot[dc][:, do * 128:(do + 1) * 128],
                                ot[dc][:], start=(dc == 0), stop=(dc == NDC - 1))
                        pp = fr.tile([128, M], BF16, name=f"pp{ro}_{do}", tag="pp")
                        nc.scalar.copy(pp[:], ps[:])
                        pst = frp.tile([M, 128], BF16, name="psT", tag="psT")
                        nc.tensor.transpose(pst[:], pp[:], eye[:])
                        nc.scalar.copy(pcat[b][ro:ro + 64, do * 128:(do + 1) * 128],
                                       pst[:])

        # FFN weights arrive while decomp1 runs
        w1t = ffnw.tile([128, NDC, DFF], FP8, name="w1t")
        nc.sync.dma_start(out=w1t[:], in_=W1T[:])
        w2t = ffnw.tile([128, NFF, D], FP8, name="w2t")
        nc.sync.dma_start(out=w2t[:], in_=W2T[:])

        # ---------- iDFT + decomp1 late combine ----------
        # per (b,dc,t4): psy=(y + x) ; ps13=(Y13'+m13x) ; ps25=(Y25'+m25x)
        # u = copy(psy) ; g=sig(u) ; h=1-g ; r = u - ps13*g - ps25*h
        dl_cm = tc.tile_pool(name="dl", bufs=2)
        dl = dl_cm.__enter__()
        psy_cm = tc.tile_pool(name="psy", bufs=2, space="PSUM")
        psyp = psy_cm.__enter__()
        for b in range(BLOC):
            for dc in range(NDC):
                dcb = slice(dc * 128, (dc + 1) * 128)
                for t4 in range(NTC):
                    ts_ = slice(t4 * 512, (t4 + 1) * 512)
                    mts = mt[b][dc][:, D0 + t4 * 512: D0 + (t4 + 1) * 512]
                    psy = psyp.tile([128, 512], F32, name="psy", tag="psy")
                    nc.tensor.matmul(psy[:], pcat[b][:, dcb], c2s2[:, ts_],
                                     start=True, stop=False)
                    nc.tensor.matmul(psy[:], eye[:], mts,
                                     start=False, stop=True)
                    ps13 = psyp.tile([128, 512], F32, name="ps13", tag="ps13")
                    nc.tensor.matmul(ps13[:], pcat[b][:, dcb], c13[:, ts_],
                                     start=True, stop=False)
                    nc.tensor.matmul(ps13[:], eye[:], m13x[b][dc][:, ts_],
                                     start=False, stop=True)
                    ps25 = psyp.tile([128, 512], F32, name="ps25", tag="ps25")
                    nc.tensor.matmul(ps25[:], pcat[b][:, dcb], c25[:, ts_],
                                     start=True, stop=False)
                    nc.tensor.matmul(ps25[:], eye[:], m25x[b][dc][:, ts_],
                                     start=False, stop=True)
                    # element combine: r = u - ma25 - g*(ma13 - ma25)
                    # (u = psy stays in PSUM; sigmoid/subs read it directly)
                    gt = dl.tile([128, 512], BF16, name="gt", tag="gt")
                    m2 = dl.tile([128, 512], BF16, name="m2", tag="m2")
                    dx = dl.tile([128, 512], BF16, name="dx", tag="dx")
                    ft = dl.tile([128, 512], BF16, name="ft", tag="ft")
                    nc.scalar.activation(gt[:], psy[:], AF.Sigmoid,
                                         scale=decs[:, 0:1], bias=decs[:, 1:2])
                    nc.scalar.copy(m2[:], ps25[:])                   # ma25 (bf16)
                    nc.vector.tensor_tensor(dx[:], ps13[:], m2[:], OP.subtract)
                    nc.vector.tensor_tensor(dx[:], dx[:], gt[:], OP.mult)
                    nc.vector.tensor_tensor(ft[:], psy[:], m2[:], OP.subtract)
                    nc.gpsimd.tensor_tensor(mts, ft[:], dx[:], OP.subtract)
        psy_cm.__exit__(None, None, None)
        dl_cm.__exit__(None, None, None)
        wpk_cm.__exit__(None, None, None)
        fr_cm.__exit__(None, None, None)

        # ---------- FFN (fp8 DoubleRow) + decomp2 ----------
        # Engine plan: FFN(b0) element ops on DVE; decomp2(b0) on DVE+ACT
        # (issued between the two FFN batches, overlapping FFN(b1) on PE);
        # FFN(b1) element ops on ACT+Pool; decomp2(b1) split DVE+Pool.
        def pass2(b, dc, sidx):
            """v (mt, padded bf16) -> series-decomp residual -> f16 stage -> DMA."""
            u = mt[b][dc]
            # refresh replicate pads from v (fp32 edge columns for the scalar op)
            ec = ear.tile([128, 2], F32, name="ec", tag=f"ec{sidx % 2}")
            nc.vector.tensor_copy(ec[:, 0:1], u[:, D0:D0 + 1])
            nc.vector.tensor_copy(ec[:, 1:2], u[:, D0 + L - 1:D0 + L])
            nc.vector.tensor_scalar_mul(u[:, 0:D0], ones13[:], ec[:, 0:1])
            nc.vector.tensor_scalar_mul(u[:, D0 + L:D0 + L + PADR],
                                        ones13[:, 0:PADR], ec[:, 1:2])
            cs = cs_tile(sidx)
            nc.vector.tensor_tensor_scan(cs[:], u[:], u[:], 0.0, OP.add, OP.bypass)
            # reuse m-tile storage of this (b,dc) + the sibling batch's tiles
            d13 = ear.tile([128, L], BF16, name="d13", tag=f"m13x{b}{dc}")
            m25 = ear.tile([128, L], BF16, name="m25", tag=f"m25x{b}{dc}")
            ob = 1 - b
            gt = ear.tile([128, L], BF16, name="gt2", tag=f"m13x{ob}{dc}")
            ft = ear.tile([128, L], BF16, name="ft2", tag=f"m25x{ob}{dc}")
            scr = scr_tile(sidx)
            nc.vector.tensor_tensor(d13[:], cs[:, 19:2067], cs[:, 6:2054],
                                    OP.subtract)
            nc.gpsimd.tensor_tensor(scr[:], cs[:, 25:2073], cs[:, 0:2048],
                                    OP.subtract)
            nc.scalar.mul(m25[:], scr[:], 1.0 / 25.0)
            ud = u[:, D0:D0 + L]
            nc.scalar.activation(gt[:], ud, AF.Sigmoid,
                                 scale=decs[:, 2:3], bias=decs[:, 3:4])
            # r = v - m25 - g*(m13 - m25)
            nc.vector.tensor_scalar_mul(d13[:], d13[:], 1.0 / 13.0)
            nc.vector.tensor_tensor(d13[:], d13[:], m25[:], OP.subtract)
            nc.vector.tensor_tensor(d13[:], d13[:], gt[:], OP.mult)
            nc.gpsimd.tensor_tensor(ft[:], ud, m25[:], OP.subtract)
            stg = stage_tile(sidx)
            nc.vector.tensor_tensor(stg[:], ft[:], d13[:], OP.subtract)
            nc.sync.dma_start(out=OUT_T[b, dc * 128:(dc + 1) * 128, :],
                              in_=stg[:])

        with tc.tile_pool(name="ffa", bufs=2) as ffa, \
             tc.tile_pool(name="ffb", bufs=1) as ffb, \
             tc.tile_pool(name="gqp", bufs=1) as gqp, \
             tc.tile_pool(name="tmq", bufs=2) as tmq, \
             tc.tile_pool(name="pshp", bufs=3, space="PSUM") as pshp, \
             tc.tile_pool(name="psfp", bufs=1, space="PSUM") as psfp:
            r1b1 = [ffb.tile([128, NDC, 512], FP8, name=f"r1b1_{t4}",
                             tag=f"r1b1_{t4}") for t4 in range(NTC)]

            def ffn_t4(b, t4):
                if b == 0:
                    r1c = ffa.tile([128, NDC, 512], FP8, name="r1c", tag="r1c")
                    for dc in range(NDC):
                        src = mt[b][dc][:, D0 + t4 * 512: D0 + (t4 + 1) * 512]
                        nc.vector.tensor_copy(r1c[:, dc, :], src)
                else:
                    r1c = r1b1[t4]
                gq = gqp.tile([128, NFF, 512], FP8, name="gq", tag="gq")
                for ff in range(NFF):
                    psh = pshp.tile([128, 512], F32, name="psH", tag="psH")
                    nc.tensor.matmul(psh[:], w1t[:, 0:2, ff * 128:(ff + 1) * 128],
                                     r1c[:, 0:2, :], start=True, stop=False,
                                     perf_mode=DR)
                    nc.tensor.matmul(psh[:], w1t[:, 2:4, ff * 128:(ff + 1) * 128],
                                     r1c[:, 2:4, :], start=False, stop=True,
                                     perf_mode=DR)
                    nc.scalar.activation(gq[:, ff, :], psh[:], AF.Gelu,
                                         scale=1.0 / FFNS)
                psf = [psfp.tile([128, 512], F32, name=f"psF{do}", tag=f"psF{do}")
                       for do in range(NDC)]
                for do in range(NDC):
                    for sp in range(0, NFF, 2):
                        nc.tensor.matmul(
                            psf[do][:], w2t[:, sp:sp + 2, do * 128:(do + 1) * 128],
                            gq[:, sp:sp + 2, :], start=(sp == 0),
                            stop=(sp == NFF - 2), perf_mode=DR)
                for do in range(NDC):
                    sl = mt[b][do][:, D0 + t4 * 512: D0 + (t4 + 1) * 512]
                    if b == 0:
                        nc.vector.scalar_tensor_tensor(
                            sl, psf[do][:], 1.0 / FFNS, sl, OP.mult, OP.add)
                    else:
                        # ACT drains psf, Pool adds (keeps DVE on decomp2(b0))
                        tm = tmq.tile([128, 512], BF16, name="tm", tag="tm")
                        nc.scalar.mul(tm[:], psf[do][:], 1.0 / FFNS)
                        nc.gpsimd.tensor_tensor(sl, sl, tm[:], OP.add)

            for t4 in range(NTC):
                ffn_t4(0, t4)
            # batch-1 FFN inputs cast on ACT (keeps DVE free for decomp2(b0))
            for t4 in range(NTC):
                for dc in range(NDC):
                    src = mt[1][dc][:, D0 + t4 * 512: D0 + (t4 + 1) * 512]
                    nc.scalar.copy(r1b1[t4][:, dc, :], src)
            # interleave: FFN(b1) on PE/ACT/Pool while decomp2(b0) runs on DVE
            for t4 in range(NTC):
                ffn_t4(1, t4)
                pass2(0, t4, t4)
            for dc in range(NDC):
                pass2(1, dc, NDC + dc)

        ffnw_cm.__exit__(None, None, None)
        ear_cm.__exit__(None, None, None)
        main_cm.__exit__(None, None, None)
        cst_cm.__exit__(None, None, None)

    if fix:
        _fix_sync_waits(nc)
    return nc


def _host_prep(inputs):
    import ml_dtypes
    bf16 = ml_dtypes.bfloat16
    fp8 = ml_dtypes.float8_e4m3
    x = np.asarray(inputs["x"], np.float32)
    bo = np.asarray(inputs["bo"], np.float32)
    modes = np.asarray(inputs["mode_index"]).astype(np.int64)
    l = np.arange(L, dtype=np.float64)
    ang = 2.0 * np.pi * np.outer(l, modes.astype(np.float64)) / L
    FC = np.concatenate([np.cos(ang), -np.sin(ang)], axis=1)          # [L, 128]
    m_out = np.arange(M, dtype=np.float64)
    w = np.where(m_out == 0, 1.0, 2.0) / L
    ang2 = 2.0 * np.pi * np.outer(m_out, l) / L
    C2 = np.concatenate([w[:, None] * np.cos(ang2),
                         w[:, None] * -np.sin(ang2)], axis=0)         # [128, L]
    C2 = C2 / WPKSH                                      # fp8 WPK compensation
    # replicate-clamped window sums of C2 (the y-side of decomp1 split)
    idx = np.arange(L)
    C13w = np.zeros_like(C2)
    for j in range(-6, 7):
        C13w += C2[:, np.clip(idx + j, 0, L - 1)]
    C13w /= 13.0
    C25w = np.zeros_like(C2)
    for j in range(-12, 13):
        C25w += C2[:, np.clip(idx + j, 0, L - 1)]
    C25w /= 25.0

    FCT = FC.reshape(NLC, 128, 128).transpose(1, 0, 2).reshape(128, NLC * 128)

    wr = np.asarray(inputs["four_wr"], np.float64)   # [H, E, O, M]
    wi = np.asarray(inputs["four_wi"], np.float64)
    wpk = np.zeros((H, M, 128, 128), np.float64)
    wpk[:, :, 0:64, 0:64] = wr.transpose(0, 3, 1, 2)
    wpk[:, :, 0:64, 64:128] = wi.transpose(0, 3, 1, 2)
    wpk[:, :, 64:128, 0:64] = -wi.transpose(0, 3, 1, 2)
    wpk[:, :, 64:128, 64:128] = wr.transpose(0, 3, 1, 2)
    WPKh = (wpk.transpose(0, 2, 1, 3).reshape(H, 128, M * 128)) * WPKSH

    dec1_w = np.asarray(inputs["dec1_w"], np.float64)
    dec1_b = np.asarray(inputs["dec1_b"], np.float64)
    dec2_w = np.asarray(inputs["dec2_w"], np.float64)
    dec2_b = np.asarray(inputs["dec2_b"], np.float64)
    decs = np.zeros((128, 4), np.float32)
    decs[:, 0] = dec1_w[0] - dec1_w[1]
    decs[:, 1] = dec1_b[0] - dec1_b[1]
    decs[:, 2] = dec2_w[0] - dec2_w[1]
    decs[:, 3] = dec2_b[0] - dec2_b[1]

    bq = np.asarray(inputs["bq"], np.float32)
    zero_pos = np.nonzero(modes == 0)[0]
    need_bq = bool(len(zero_pos)) and bool(np.any(bq != 0))
    j0 = int(zero_pos[0]) if need_bq else 0
    BQ4 = np.ascontiguousarray((L * bq).reshape(NDC, 128).T).astype(np.float32)

    # FFN weights: [128, S, F] fp8 with k-subtile interleave, x16
    w1 = np.asarray(inputs["conv1_w"], np.float32)   # [DFF, D]
    w2 = np.asarray(inputs["conv2_w"], np.float32)   # [D, DFF]
    W1T = (w1.T.reshape(NDC, 128, DFF) * FFNS).astype(fp8)          # [s,p,f]
    W1T = np.ascontiguousarray(W1T.transpose(1, 0, 2))              # [128,s,f]
    W2T = (w2.T.reshape(NFF, 128, D) * FFNS).astype(fp8)
    W2T = np.ascontiguousarray(W2T.transpose(1, 0, 2))

    shared = {
        "FCT": FCT.astype(fp8),
        "C2S2": C2.astype(bf16),
        "C13": C13w.astype(bf16),
        "C25": C25w.astype(bf16),
        "WQT": np.ascontiguousarray(np.asarray(inputs["Wq"], np.float32).T).astype(bf16),
        "WOT": np.ascontiguousarray(np.asarray(inputs["Wo"], np.float32).T).astype(bf16),
        "WPK": WPKh.astype(fp8),
        "W1T": W1T, "W2T": W2T,
        "EYE": np.eye(128, dtype=np.float32).astype(bf16),
        "BQ4": BQ4, "DECS": decs,
    }
    in_maps = []
    for c in range(NC_):
        xl = x[c * BLOC:(c + 1) * BLOC]                       # [2, L, D]
        xt = (xl + bo[None, None, :]).transpose(0, 2, 1)      # [2, D, L]
        xtp = np.zeros((BLOC, D, LP), np.float32)
        xtp[:, :, D0:D0 + L] = xt
        xtp[:, :, 0:D0] = xt[:, :, 0:1]
        xtp[:, :, D0 + L:D0 + L + PADR] = xt[:, :, L - 1:L]
        xbf = xl.astype(fp8)                                  # [2, L, D]
        XBFc = np.ascontiguousarray(
            xbf.reshape(BLOC, NLC, 128, D).transpose(0, 2, 1, 3)
        ).reshape(BLOC, 128, NLC * D)
        im = dict(shared)
        im["XTB"] = xtp.astype(bf16)
        im["XBF"] = XBFc
        in_maps.append(im)
    return in_maps, need_bq, j0


def kernel(**inputs):
    from concourse.bass_utils import run_bass_kernel_spmd

    in_maps, need_bq, j0 = _host_prep(inputs)
    key = (need_bq, j0)
    if key not in _prog_cache:
        _prog_cache[key] = _build_program(need_bq, j0)
    nc = _prog_cache[key]
    res = run_bass_kernel_spmd(nc, in_maps, core_ids=list(range(NC_)))
    outs = []
    for c in range(NC_):
        ot = np.asarray(res.results[c]["OUT_T"])              # [2, D, L]
        outs.append(np.ascontiguousarray(ot.transpose(0, 2, 1)))
    return np.concatenate(outs, axis=0).astype(np.float32)


# revision 53
# speedup vs baseline: 1.2583x; 1.0076x over previous
"""FEDformer encoder layer on 8 TRN2 NeuronCores — batch-data-parallel Bass kernel.

Strategy (self-contained; shapes hardcoded):
  B=16,L=2048,D=512,H=8,E=64,M=64,DFF=2048; 8 cores x 2 batches each; no collectives.

  Math restructuring (validated against the jax reference):
   - rfft+mode-gather == x @ Fcat where Fcat[l, 0:64]=cos(2*pi*k_j*l/L),
     Fcat[l, 64:128]=-sin(...), k_j = mode_index.
   - Wq/Wo commute with the DFT -> applied in mode space. k/v projections are
     dead code in the reference.
   - irfft of a spectrum with only the selected modes == P @ C2S2.
   - The Fourier branch contributes ~1e-5 absolute to an O(1) output, so the
     whole branch runs in fp8/bf16 (WPK pre-scaled by 2^17 on host; 2^-17
     folded into the iDFT matrices).
   - series-decomp: K=2 softmax == sigmoid of weight/bias deltas; moving
     averages via fp32 cumsum over a replicate-padded tile + shifted
     subtracts (pads baked into the padded layout; no edge fixups).
   - decomp1 split trick: u = (x+bo) + y with y = pcat @ C2S2 linear, so
     S13(u) = S13(x+bo) + pcat @ C13 (C13 = window-summed C2S2, host-made).
     The x-side scans/diffs run at kernel start, hiding the WPK DMA.
   - FFN in fp8e4 DoubleRow (weights x16 host-side; 1/16 folded into the
     gelu input scale and the final residual add).
   - bo folded into the host-prepared x (XTB = (x+bo)^T).

  Layout: device works feature-major ([D, Lpad]) in bf16; token-major fp8
  copy (XBF) only for the DFT.
"""

import numpy as np

B, L, D, H, M, DFF = 16, 2048, 512, 8, 64, 2048
E = D // H
NC_ = 8
BLOC = B // NC_          # batches per core
MEXT = 2 * M             # re|im rows
NDC = D // 128           # 4 feature tiles
NFF = DFF // 128         # 16 dff tiles
NLC = L // 128           # 16 token chunks of 128
NTC = L // 512           # 4 token chunks of 512
PADL = 13                # left replicate pad (cumsum needs one extra)
PADR = 12
LP = 2080                # PADL + L + PADR + 7 spare zeros
D0 = PADL                # data column offset in padded tiles
WPKSH = float(2 ** 17)   # fp8 scale for Fourier weights
FFNS = 16.0              # fp8 scale for FFN weights

_prog_cache = {}
_fixn = [0]


def _fix_sync_waits(nc, max_waits=1, max_updates=4):
    """Split >max sem-waits/updates per instruction onto adjacent nops.

    The AWS neuronx-cc walrus rejects instructions carrying too many sync
    commands ("Too many sync wait commands"); Tile's tail drain aggregates one
    wait per outstanding semaphore. Engine-order execution makes the split
    semantically identical.
    """
    import concourse.mybir as mybir

    for f in nc.m.functions:
        for bb in f.blocks:
            insts = bb.instructions
            i = 0
            while i < len(insts):
                ins = insts[i]
                si = ins.sync_info
                if si is not None and si.on_wait and len(si.on_wait) > max_waits:
                    waits = list(si.on_wait)
                    si.on_wait = waits[-max_waits:]
                    rest = waits[:-max_waits]
                    chunks = [rest[j:j + max_waits]
                              for j in range(0, len(rest), max_waits)]
                    for c in reversed(chunks):
                        _fixn[0] += 1
                        nop = mybir.InstNoOp(name=f"I-fixw-{_fixn[0]}", ins=[], outs=[])
                        nop.engine = ins.engine
                        nop.sync_info = mybir.SyncInfo(on_wait=c, on_update=[])
                        insts.insert(i, nop)
                        i += 1
                if si is not None and si.on_update and len(si.on_update) > max_updates:
                    ups = list(si.on_update)
                    si.on_update = ups[:max_updates]
                    rest = ups[max_updates:]
                    chunks = [rest[j:j + max_updates]
                              for j in range(0, len(rest), max_updates)]
                    for c in chunks:
                        _fixn[0] += 1
                        nop = mybir.InstNoOp(name=f"I-fixu-{_fixn[0]}", ins=[], outs=[])
                        nop.engine = ins.engine
                        nop.sync_info = mybir.SyncInfo(on_wait=[], on_update=c)
                        insts.insert(i + 1, nop)
                        i += 1
                i += 1


def _build_program(need_bq, j0, fix=True):
    import concourse.bass as bass
    import concourse.mybir as mybir
    from concourse.tile import TileContext

    F32 = mybir.dt.float32
    BF16 = mybir.dt.bfloat16
    FP8 = mybir.dt.float8e4
    AF = mybir.ActivationFunctionType
    OP = mybir.AluOpType
    DR = mybir.MatmulPerfMode.DoubleRow

    nc = bass.Bass()

    # ---- DRAM I/O ----
    XTB = nc.dram_tensor("XTB", [BLOC, D, LP], BF16, kind="ExternalInput")
    XBF = nc.dram_tensor("XBF", [BLOC, 128, NLC * D], FP8, kind="ExternalInput")
    FCT = nc.dram_tensor("FCT", [128, NLC * 128], FP8, kind="ExternalInput")
    C2S2 = nc.dram_tensor("C2S2", [128, L], BF16, kind="ExternalInput")
    C13 = nc.dram_tensor("C13", [128, L], BF16, kind="ExternalInput")
    C25 = nc.dram_tensor("C25", [128, L], BF16, kind="ExternalInput")
    WQT = nc.dram_tensor("WQT", [D, D], BF16, kind="ExternalInput")
    WOT = nc.dram_tensor("WOT", [D, D], BF16, kind="ExternalInput")
    WPK = nc.dram_tensor("WPK", [H, 128, M * 128], FP8, kind="ExternalInput")
    W1T = nc.dram_tensor("W1T", [128, NDC, DFF], FP8, kind="ExternalInput")
    W2T = nc.dram_tensor("W2T", [128, NFF, D], FP8, kind="ExternalInput")
    EYE = nc.dram_tensor("EYE", [128, 128], BF16, kind="ExternalInput")
    BQ4 = nc.dram_tensor("BQ4", [128, NDC], F32, kind="ExternalInput")
    DECS = nc.dram_tensor("DECS", [128, 4], F32, kind="ExternalInput")
    F16 = mybir.dt.float16
    OUT_T = nc.dram_tensor("OUT_T", [BLOC, D, L], F16, kind="ExternalOutput")

    with TileContext(nc) as tc:
        # ---------- persistent pools (LIFO: wpkp/fr close after fourier,
        # ffnw after the FFN, the rest at the end) ----------
        cst_cm = tc.tile_pool(name="cst", bufs=1)
        cst = cst_cm.__enter__()
        main_cm = tc.tile_pool(name="main", bufs=1)
        mainp = main_cm.__enter__()
        ear_cm = tc.tile_pool(name="ear", bufs=1)
        ear = ear_cm.__enter__()
        ffnw_cm = tc.tile_pool(name="ffnw", bufs=1)
        ffnw = ffnw_cm.__enter__()
        fr_cm = tc.tile_pool(name="fr", bufs=1)
        fr = fr_cm.__enter__()

        # DFT inputs first (DFT is the head of the dependency chain), then the
        # first WPK chunks (mode-mix stream), then x, then later-used consts.
        fct = cst.tile([128, NLC * 128], FP8, name="fct")
        nc.sync.dma_start(out=fct[:], in_=FCT[:])
        xbfs = [cst.tile([128, NLC * D], FP8, name=f"xbf{b}", tag=f"xbf{b}")
                for b in range(BLOC)]
        for b in range(BLOC):
            nc.sync.dma_start(out=xbfs[b][:], in_=XBF[b])
        wqt = [cst.tile([128, D], BF16, name=f"wqt{i}") for i in range(NDC)]
        for i in range(NDC):
            nc.sync.dma_start(out=wqt[i][:], in_=WQT[i * 128:(i + 1) * 128, :])

        # WPK stream: 2 half-head chunks per head (descriptor generation is
        # ~1us serial per dma_start, so few big DMAs beat many small ones)
        wpk_cm = tc.tile_pool(name="wpkp", bufs=3)
        wpkp = wpk_cm.__enter__()

        def wpk_chunk(h, hf):
            t_ = wpkp.tile([128, 32 * 128], FP8, name=f"wpk{h}_{hf}", tag="wpk")
            nc.sync.dma_start(out=t_[:],
                              in_=WPK[h][:, hf * 4096:(hf + 1) * 4096])
            return t_

        wpk_pre = [wpk_chunk(0, 0), wpk_chunk(0, 1), wpk_chunk(1, 0)]

        # main activation tiles: (x+bo) -> u -> r1 -> v, in place, bf16 padded
        mt = [[mainp.tile([128, LP], BF16, name=f"m_{b}_{dc}")
               for dc in range(NDC)] for b in range(BLOC)]
        for b in range(BLOC):
            for dc in range(NDC):
                nc.sync.dma_start(out=mt[b][dc][:],
                                  in_=XTB[b, dc * 128:(dc + 1) * 128, :])

        wot = [cst.tile([128, D], BF16, name=f"wot{i}") for i in range(NDC)]
        for i in range(NDC):
            nc.sync.dma_start(out=wot[i][:], in_=WOT[i * 128:(i + 1) * 128, :])
        # c2s2/c13/c25 DMAs are issued after the mode-mix loop (they are only
        # needed at the iDFT and would delay the WPK stream here)
        c2s2 = cst.tile([128, L], BF16, name="c2s2")
        c13 = cst.tile([128, L], BF16, name="c13")
        c25 = cst.tile([128, L], BF16, name="c25")
        eye = cst.tile([128, 128], BF16, name="eye")
        nc.sync.dma_start(out=eye[:], in_=EYE[:])
        decs = cst.tile([128, 4], F32, name="decs")
        nc.sync.dma_start(out=decs[:], in_=DECS[:])
        ones13 = cst.tile([128, PADL], BF16, name="ones13")
        nc.vector.memset(ones13[:], 1.0)
        bq4 = None
        if need_bq:
            bq4 = cst.tile([128, NDC], F32, name="bq4")
            nc.sync.dma_start(out=bq4[:], in_=BQ4[:])

        # early pool tiles: rotating cumsums + per-tile windowed sums
        NCS = 2
        def cs_tile(i):
            return ear.tile([128, LP], F32, name="cs", tag=f"cs{i % NCS}")
        def stage_tile(i):
            return ear.tile([128, L], F16, name="stg", tag="stg0")
        def scr_tile(i):
            return ear.tile([128, L], F32, name="scr", tag="scr0")
        m13x = [[ear.tile([128, L], BF16, name=f"m13x{b}{dc}", tag=f"m13x{b}{dc}")
                 for dc in range(NDC)] for b in range(BLOC)]
        m25x = [[ear.tile([128, L], BF16, name=f"m25x{b}{dc}", tag=f"m25x{b}{dc}")
                 for dc in range(NDC)] for b in range(BLOC)]

        # ---------- early: scans + windowed diffs of (x+bo), fills the ----
        # ---------- window where the PE waits on the WPK weight stream ----
        csi = 0
        for b in range(BLOC):
            for dc in range(NDC):
                cs = cs_tile(csi)
                scr = scr_tile(csi)
                csi += 1
                u = mt[b][dc]
                nc.vector.tensor_tensor_scan(cs[:], u[:], u[:], 0.0,
                                             OP.add, OP.bypass)
                # S13(t) = cs[t+19] - cs[t+6]; S25(t) = cs[t+25] - cs[t]
                nc.vector.tensor_tensor(m13x[b][dc][:], cs[:, 19:2067],
                                        cs[:, 6:2054], OP.subtract)
                nc.scalar.mul(m13x[b][dc][:], m13x[b][dc][:], 1.0 / 13.0)
                nc.gpsimd.tensor_tensor(scr[:], cs[:, 25:2073],
                                        cs[:, 0:2048], OP.subtract)
                nc.scalar.mul(m25x[b][dc][:], scr[:], 1.0 / 25.0)

        # ---------- Fourier branch (fp8/bf16) ----------
        with tc.tile_pool(name="frp", bufs=2, space="PSUM") as frp:
            qt = [[None] * NDC for _ in range(BLOC)]
            for b in range(BLOC):
                xbf = xbfs[b]
                # DFT: xselT[d, m-ext] = sum_l x[l, d] * Fcat[l, m-ext]
                xselT = fr.tile([128, NDC * 128], BF16, name=f"xselT{b}",
                                tag=f"xselT{b}")
                for dc in range(NDC):
                    ps = frp.tile([128, 128], F32, name="psA", tag="psA")
                    for lc in range(NLC):
                        nc.tensor.matmul(
                            ps[:],
                            xbf[:, lc * D + dc * 128: lc * D + (dc + 1) * 128],
                            fct[:, lc * 128:(lc + 1) * 128],
                            start=(lc == 0), stop=(lc == NLC - 1))
                    nc.scalar.copy(xselT[:, dc * 128:(dc + 1) * 128], ps[:])
                # q-projection in mode space (fp8 out for the mode mix)
                for do in range(NDC):
                    qt[b][do] = fr.tile([128, 128], FP8, name=f"qt{b}_{do}",
                                        tag=f"qt{b}_{do}")
                    ps = frp.tile([128, 128], F32, name="psQ", tag="psA")
                    for dc in range(NDC):
                        nc.tensor.matmul(
                            ps[:], wqt[dc][:, do * 128:(do + 1) * 128],
                            xselT[:, dc * 128:(dc + 1) * 128],
                            start=(dc == 0), stop=(dc == NDC - 1))
                    if need_bq:
                        nc.vector.tensor_tensor(
                            ps[:, j0:j0 + 1], ps[:, j0:j0 + 1],
                            bq4[:, do:do + 1], OP.add)
                    nc.scalar.copy(qt[b][do][:], ps[:])

            # mode mix: RH_h rows 0:64 = Qre, 64:128 = Qim; col = 2m + b
            rh = [fr.tile([128, 128], FP8, name=f"rh{h}", tag=f"rh{h}")
                  for h in range(H)]
            for h in range(H):
                src_do, r0 = h // 2, (h % 2) * 64
                for b in range(BLOC):
                    rhv = rh[h].rearrange("p (m t) -> p m t", t=2)
                    nc.scalar.copy(rhv[0:64, :, b], qt[b][src_do][r0:r0 + 64, 0:64])
                    nc.scalar.copy(rhv[64:128, :, b], qt[b][src_do][r0:r0 + 64, 64:128])
            otre = [[fr.tile([128, M], BF16, name=f"otre{b}_{dc}", tag=f"otre{b}{dc}")
                     for dc in range(NDC)] for b in range(BLOC)]
            otim = [[fr.tile([128, M], BF16, name=f"otim{b}_{dc}", tag=f"otim{b}{dc}")
                     for dc in range(NDC)] for b in range(BLOC)]
            npre = len(wpk_pre)
            for h in range(H):
                psm = frp.tile([128, 128], F32, name="psM", tag="psM")
                for hf in range(2):
                    ci = 2 * h + hf
                    wpk_q = wpk_pre[ci] if ci < npre else wpk_chunk(h, hf)
                    for mq in range(32):
                        m = hf * 32 + mq
                        nc.tensor.matmul(
                            psm[:, 2 * m:2 * m + 2],
                            wpk_q[:, mq * 128:(mq + 1) * 128],
                            rh[h][:, 2 * m:2 * m + 2],
                            start=True, stop=True)
                psv = psm.rearrange("p (m t) -> p m t", t=2)
                dc, r0 = h // 2, (h % 2) * 64
                for b in range(BLOC):
                    nc.scalar.copy(otre[b][dc][r0:r0 + 64, :], psv[0:64, :, b])
                    nc.scalar.copy(otim[b][dc][r0:r0 + 64, :], psv[64:128, :, b])
                if h == H - 1:
                    nc.sync.dma_start(out=c2s2[:], in_=C2S2[:])
                    nc.sync.dma_start(out=c13[:], in_=C13[:])
                    nc.sync.dma_start(out=c25[:], in_=C25[:])

            # Wo projection in mode space, then transpose into pcat_b
            pcat = [fr.tile([128, D], BF16, name=f"pcat{b}", tag=f"pcat{b}")
                    for b in range(BLOC)]
            for b in range(BLOC):
                for ro, ot in ((0, otre[b]), (64, otim[b])):
                    for do in range(NDC):
                        ps = frp.tile([128, M], F32, name="psP", tag="psP")
                        for dc in range(NDC):
                            nc.tensor.matmul(
                                ps[:], wot[dc][:, do * 128:(do + 1) * 128],
                                ot[dc][:], start=(dc == 0), stop=(dc == NDC - 1))
                        pp = fr.tile([128, M], BF16, name=f"pp{ro}_{do}", tag="pp")
                        nc.scalar.copy(pp[:], ps[:])
                        pst = frp.tile([M, 128], BF16, name="psT", tag="psT")
                        nc.tensor.transpose(pst[:], pp[:], eye[:])
                        nc.scalar.copy(pcat[b][ro:ro + 64, do * 128:(do + 1) * 128],
                                       pst[:])

        # FFN weights arrive while decomp1 runs
        w1t = ffnw.tile([128, NDC, DFF], FP8, name="w1t")
        nc.sync.dma_start(out=w1t[:], in_=W1T[:])
        w2t = ffnw.tile([128, NFF, D], FP8, name="w2t")
        nc.sync.dma_start(out=w2t[:], in_=W2T[:])

        # ---------- iDFT + decomp1 late combine ----------
        # per (b,dc,t4): psy=(y + x) ; ps13=(Y13'+m13x) ; ps25=(Y25'+m25x)
        # u = copy(psy) ; g=sig(u) ; h=1-g ; r = u - ps13*g - ps25*h
        dl_cm = tc.tile_pool(name="dl", bufs=2)
        dl = dl_cm.__enter__()
        psy_cm = tc.tile_pool(name="psy", bufs=2, space="PSUM")
        psyp = psy_cm.__enter__()
        for b in range(BLOC):
            for dc in range(NDC):
                dcb = slice(dc * 128, (dc + 1) * 128)
                for t4 in range(NTC):
                    ts_ = slice(t4 * 512, (t4 + 1) * 512)
                    mts = mt[b][dc][:, D0 + t4 * 512: D0 + (t4 + 1) * 512]
                    psy = psyp.tile([128, 512], F32, name="psy", tag="psy")
                    nc.tensor.matmul(psy[:], pcat[b][:, dcb], c2s2[:, ts_],
                                     start=True, stop=False)
                    nc.tensor.matmul(psy[:], eye[:], mts,
                                     start=False, stop=True)
                    ps13 = psyp.tile([128, 512], F32, name="ps13", tag="ps13")
                    nc.tensor.matmul(ps13[:], pcat[b][:, dcb], c13[:, ts_],
                                     start=True, stop=False)
                    nc.tensor.matmul(ps13[:], eye[:], m13x[b][dc][:, ts_],
                                     start=False, stop=True)
                    ps25 = psyp.tile([128, 512], F32, name="ps25", tag="ps25")
                    nc.tensor.matmul(ps25[:], pcat[b][:, dcb], c25[:, ts_],
                                     start=True, stop=False)
                    nc.tensor.matmul(ps25[:], eye[:], m25x[b][dc][:, ts_],
                                     start=False, stop=True)
                    # element combine: r = u - ma25 - g*(ma13 - ma25)
                    # (u = psy stays in PSUM; sigmoid/subs read it directly)
                    gt = dl.tile([128, 512], BF16, name="gt", tag="gt")
                    m2 = dl.tile([128, 512], BF16, name="m2", tag="m2")
                    dx = dl.tile([128, 512], BF16, name="dx", tag="dx")
                    ft = dl.tile([128, 512], BF16, name="ft", tag="ft")
                    nc.scalar.activation(gt[:], psy[:], AF.Sigmoid,
                                         scale=decs[:, 0:1], bias=decs[:, 1:2])
                    nc.scalar.copy(m2[:], ps25[:])                   # ma25 (bf16)
                    nc.vector.tensor_tensor(dx[:], ps13[:], m2[:], OP.subtract)
                    nc.vector.tensor_tensor(dx[:], dx[:], gt[:], OP.mult)
                    nc.vector.tensor_tensor(ft[:], psy[:], m2[:], OP.subtract)
                    nc.gpsimd.tensor_tensor(mts, ft[:], dx[:], OP.subtract)
        psy_cm.__exit__(None, None, None)
        dl_cm.__exit__(None, None, None)
        wpk_cm.__exit__(None, None, None)
        fr_cm.__exit__(None, None, None)

        # ---------- FFN (fp8 DoubleRow) + decomp2 ----------
        # Engine plan: FFN(b0) element ops on DVE; decomp2(b0) on DVE+ACT
        # (issued between the two FFN batches, overlapping FFN(b1) on PE);
        # FFN(b1) element ops on ACT+Pool; decomp2(b1) split DVE+Pool.
        def pass2(b, dc, sidx):
            """v (mt, padded bf16) -> series-decomp residual -> f16 stage -> DMA."""
            u = mt[b][dc]
            # refresh replicate pads from v (fp32 edge columns for the scalar op)
            ec = ear.tile([128, 2], F32, name="ec", tag=f"ec{sidx % 2}")
            nc.vector.tensor_copy(ec[:, 0:1], u[:, D0:D0 + 1])
            nc.vector.tensor_copy(ec[:, 1:2], u[:, D0 + L - 1:D0 + L])
            nc.vector.tensor_scalar_mul(u[:, 0:D0], ones13[:], ec[:, 0:1])
            nc.vector.tensor_scalar_mul(u[:, D0 + L:D0 + L + PADR],
                                        ones13[:, 0:PADR], ec[:, 1:2])
            cs = cs_tile(sidx)
            nc.vector.tensor_tensor_scan(cs[:], u[:], u[:], 0.0, OP.add, OP.bypass)
            # reuse m-tile storage of this (b,dc) + the sibling batch's tiles
            d13 = ear.tile([128, L], BF16, name="d13", tag=f"m13x{b}{dc}")
            m25 = ear.tile([128, L], BF16, name="m25", tag=f"m25x{b}{dc}")
            ob = 1 - b
            gt = ear.tile([128, L], BF16, name="gt2", tag=f"m13x{ob}{dc}")
            ft = ear.tile([128, L], BF16, name="ft2", tag=f"m25x{ob}{dc}")
            scr = scr_tile(sidx)
            nc.vector.tensor_tensor(d13[:], cs[:, 19:2067], cs[:, 6:2054],
                                    OP.subtract)
            nc.gpsimd.tensor_tensor(scr[:], cs[:, 25:2073], cs[:, 0:2048],
                                    OP.subtract)
            nc.scalar.mul(m25[:], scr[:], 1.0 / 25.0)
            ud = u[:, D0:D0 + L]
            nc.scalar.activation(gt[:], ud, AF.Sigmoid,
                                 scale=decs[:, 2:3], bias=decs[:, 3:4])
            # r = v - m25 - g*(m13 - m25)
            nc.scalar.mul(d13[:], d13[:], 1.0 / 13.0)
            nc.vector.tensor_tensor(d13[:], d13[:], m25[:], OP.subtract)
            nc.vector.tensor_tensor(d13[:], d13[:], gt[:], OP.mult)
            nc.gpsimd.tensor_tensor(ft[:], ud, m25[:], OP.subtract)
            stg = stage_tile(sidx)
            nc.vector.tensor_tensor(stg[:], ft[:], d13[:], OP.subtract)
            nc.sync.dma_start(out=OUT_T[b, dc * 128:(dc + 1) * 128, :],
                              in_=stg[:])

        with tc.tile_pool(name="ffa", bufs=2) as ffa, \
             tc.tile_pool(name="ffb", bufs=1) as ffb, \
             tc.tile_pool(name="gqp", bufs=1) as gqp, \
             tc.tile_pool(name="tmq", bufs=2) as tmq, \
             tc.tile_pool(name="pshp", bufs=2, space="PSUM") as pshp, \
             tc.tile_pool(name="psfp", bufs=1, space="PSUM") as psfp:
            r1b1 = [ffb.tile([128, NDC, 512], FP8, name=f"r1b1_{t4}",
                             tag=f"r1b1_{t4}") for t4 in range(NTC)]

            def ffn_t4(b, t4):
                if b == 0:
                    r1c = ffa.tile([128, NDC, 512], FP8, name="r1c", tag="r1c")
                    for dc in range(NDC):
                        src = mt[b][dc][:, D0 + t4 * 512: D0 + (t4 + 1) * 512]
                        nc.vector.tensor_copy(r1c[:, dc, :], src)
                else:
                    r1c = r1b1[t4]
                gq = gqp.tile([128, NFF, 512], FP8, name="gq", tag="gq")
                for fp in range(NFF // 2):   # two ff blocks per psum + gelu
                    psh = pshp.tile([128, 1024], F32, name="psH", tag="psH")
                    for k in range(2):
                        ff = 2 * fp + k
                        hs = psh[:, k * 512:(k + 1) * 512]
                        nc.tensor.matmul(hs, w1t[:, 0:2, ff * 128:(ff + 1) * 128],
                                         r1c[:, 0:2, :], start=True, stop=False,
                                         perf_mode=DR)
                        nc.tensor.matmul(hs, w1t[:, 2:4, ff * 128:(ff + 1) * 128],
                                         r1c[:, 2:4, :], start=False, stop=True,
                                         perf_mode=DR)
                    nc.scalar.activation(gq[:, 2 * fp:2 * fp + 2, :], psh[:],
                                         AF.Gelu, scale=1.0 / FFNS)
                psf = [psfp.tile([128, 512], F32, name=f"psF{do}", tag=f"psF{do}")
                       for do in range(NDC)]
                for do in range(NDC):
                    for sp in range(0, NFF, 2):
                        nc.tensor.matmul(
                            psf[do][:], w2t[:, sp:sp + 2, do * 128:(do + 1) * 128],
                            gq[:, sp:sp + 2, :], start=(sp == 0),
                            stop=(sp == NFF - 2), perf_mode=DR)
                for do in range(NDC):
                    sl = mt[b][do][:, D0 + t4 * 512: D0 + (t4 + 1) * 512]
                    if b == 0:
                        nc.vector.scalar_tensor_tensor(
                            sl, psf[do][:], 1.0 / FFNS, sl, OP.mult, OP.add)
                    else:
                        # ACT drains psf, Pool adds (keeps DVE on decomp2(b0))
                        tm = tmq.tile([128, 512], BF16, name="tm", tag="tm")
                        nc.scalar.mul(tm[:], psf[do][:], 1.0 / FFNS)
                        nc.gpsimd.tensor_tensor(sl, sl, tm[:], OP.add)

            for t4 in range(NTC):
                ffn_t4(0, t4)
            # batch-1 FFN inputs cast on ACT (keeps DVE free for decomp2(b0))
            for t4 in range(NTC):
                for dc in range(NDC):
                    src = mt[1][dc][:, D0 + t4 * 512: D0 + (t4 + 1) * 512]
                    nc.scalar.copy(r1b1[t4][:, dc, :], src)
            # interleave: FFN(b1) on PE/ACT/Pool while decomp2(b0) runs on DVE
            for t4 in range(NTC):
                ffn_t4(1, t4)
                pass2(0, t4, t4)
            for dc in range(NDC):
                pass2(1, dc, NDC + dc)

        ffnw_cm.__exit__(None, None, None)
        ear_cm.__exit__(None, None, None)
        main_cm.__exit__(None, None, None)
        cst_cm.__exit__(None, None, None)

    if fix:
        _fix_sync_waits(nc)
    return nc


def _host_prep(inputs):
    import ml_dtypes
    bf16 = ml_dtypes.bfloat16
    fp8 = ml_dtypes.float8_e4m3
    x = np.asarray(inputs["x"], np.float32)
    bo = np.asarray(inputs["bo"], np.float32)
    modes = np.asarray(inputs["mode_index"]).astype(np.int64)
    l = np.arange(L, dtype=np.float64)
    ang = 2.0 * np.pi * np.outer(l, modes.astype(np.float64)) / L
    FC = np.concatenate([np.cos(ang), -np.sin(ang)], axis=1)          # [L, 128]
    m_out = np.arange(M, dtype=np.float64)
    w = np.where(m_out == 0, 1.0, 2.0) / L
    ang2 = 2.0 * np.pi * np.outer(m_out, l) / L
    C2 = np.concatenate([w[:, None] * np.cos(ang2),
                         w[:, None] * -np.sin(ang2)], axis=0)         # [128, L]
    C2 = C2 / WPKSH                                      # fp8 WPK compensation
    # replicate-clamped window sums of C2 (the y-side of decomp1 split)
    idx = np.arange(L)
    C13w = np.zeros_like(C2)
    for j in range(-6, 7):
        C13w += C2[:, np.clip(idx + j, 0, L - 1)]
    C13w /= 13.0
    C25w = np.zeros_like(C2)
    for j in range(-12, 13):
        C25w += C2[:, np.clip(idx + j, 0, L - 1)]
    C25w /= 25.0

    FCT = FC.reshape(NLC, 128, 128).transpose(1, 0, 2).reshape(128, NLC * 128)

    wr = np.asarray(inputs["four_wr"], np.float64)   # [H, E, O, M]
    wi = np.asarray(inputs["four_wi"], np.float64)
    wpk = np.zeros((H, M, 128, 128), np.float64)
    wpk[:, :, 0:64, 0:64] = wr.transpose(0, 3, 1, 2)
    wpk[:, :, 0:64, 64:128] = wi.transpose(0, 3, 1, 2)
    wpk[:, :, 64:128, 0:64] = -wi.transpose(0, 3, 1, 2)
    wpk[:, :, 64:128, 64:128] = wr.transpose(0, 3, 1, 2)
    WPKh = (wpk.transpose(0, 2, 1, 3).reshape(H, 128, M * 128)) * WPKSH

    dec1_w = np.asarray(inputs["dec1_w"], np.float64)
    dec1_b = np.asarray(inputs["dec1_b"], np.float64)
    dec2_w = np.asarray(inputs["dec2_w"], np.float64)
    dec2_b = np.asarray(inputs["dec2_b"], np.float64)
    decs = np.zeros((128, 4), np.float32)
    decs[:, 0] = dec1_w[0] - dec1_w[1]
    decs[:, 1] = dec1_b[0] - dec1_b[1]
    decs[:, 2] = dec2_w[0] - dec2_w[1]
    decs[:, 3] = dec2_b[0] - dec2_b[1]

    bq = np.asarray(inputs["bq"], np.float32)
    zero_pos = np.nonzero(modes == 0)[0]
    need_bq = bool(len(zero_pos)) and bool(np.any(bq != 0))
    j0 = int(zero_pos[0]) if need_bq else 0
    BQ4 = np.ascontiguousarray((L * bq).reshape(NDC, 128).T).astype(np.float32)

    # FFN weights: [128, S, F] fp8 with k-subtile interleave, x16
    w1 = np.asarray(inputs["conv1_w"], np.float32)   # [DFF, D]
    w2 = np.asarray(inputs["conv2_w"], np.float32)   # [D, DFF]
    W1T = (w1.T.reshape(NDC, 128, DFF) * FFNS).astype(fp8)          # [s,p,f]
    W1T = np.ascontiguousarray(W1T.transpose(1, 0, 2))              # [128,s,f]
    W2T = (w2.T.reshape(NFF, 128, D) * FFNS).astype(fp8)
    W2T = np.ascontiguousarray(W2T.transpose(1, 0, 2))

    shared = {
        "FCT": FCT.astype(fp8),
        "C2S2": C2.astype(bf16),
        "C13": C13w.astype(bf16),
        "C25": C25w.astype(bf16),
        "WQT": np.ascontiguousarray(np.asarray(inputs["Wq"], np.float32).T).astype(bf16),
        "WOT": np.ascontiguousarray(np.asarray(inputs["Wo"], np.float32).T).astype(bf16),
        "WPK": WPKh.astype(fp8),
        "W1T": W1T, "W2T": W2T,
        "EYE": np.eye(128, dtype=np.float32).astype(bf16),
        "BQ4": BQ4, "DECS": decs,
    }
    in_maps = []
    for c in range(NC_):
        xl = x[c * BLOC:(c + 1) * BLOC]                       # [2, L, D]
        xt = (xl + bo[None, None, :]).transpose(0, 2, 1)      # [2, D, L]
        xtp = np.zeros((BLOC, D, LP), np.float32)
        xtp[:, :, D0:D0 + L] = xt
        xtp[:, :, 0:D0] = xt[:, :, 0:1]
        xtp[:, :, D0 + L:D0 + L + PADR] = xt[:, :, L - 1:L]
        xbf = xl.astype(fp8)                                  # [2, L, D]
        XBFc = np.ascontiguousarray(
            xbf.reshape(BLOC, NLC, 128, D).transpose(0, 2, 1, 3)
        ).reshape(BLOC, 128, NLC * D)
        im = dict(shared)
        im["XTB"] = xtp.astype(bf16)
        im["XBF"] = XBFc
        in_maps.append(im)
    return in_maps, need_bq, j0


def kernel(**inputs):
    from concourse.bass_utils import run_bass_kernel_spmd

    in_maps, need_bq, j0 = _host_prep(inputs)
    key = (need_bq, j0)
    if key not in _prog_cache:
        _prog_cache[key] = _build_program(need_bq, j0)
    nc = _prog_cache[key]
    res = run_bass_kernel_spmd(nc, in_maps, core_ids=list(range(NC_)))
    outs = []
    for c in range(NC_):
        ot = np.asarray(res.results[c]["OUT_T"])              # [2, D, L]
        outs.append(np.ascontiguousarray(ot.transpose(0, 2, 1)))
    return np.concatenate(outs, axis=0).astype(np.float32)
